# revision 13
# baseline (speedup 1.0000x reference)
"""Bahdanau attention kernel for Trainium2, 8-core data-parallel.

Problem (B=32, L=1024, H=1024, fp32):
    h     = tanh(q @ W1.T + b1 + v @ W2.T + b2)        # (B, L, H)
    score = h @ Vw.T + vb                              # (B, L, H)
    att   = softmax(score, axis=-1)                    # (B, L, H)
    ctx   = att @ v                                    # (B, L, H)  (bmm over kv dim)
    returns (att, ctx)

Strategy:
  - Data-parallel: 4 batches per core on 8 cores.
  - Everything on-device runs in a TRANSPOSED layout [h, l] so that the
    contraction dim (h / k) always lands on SBUF partitions and no on-device
    transposes are needed. The host pre-transposes q and value per batch
    (and pre-transposes the weight matrices), and transposes the attention
    weights output back after gathering.
  - A tunable slice of stage A (the q@W1 + v@W2 preactivation) runs in fp8e4
    with perf_mode=DoubleRow: one matmul contracts the j-th 128-block of BOTH
    streams (W1_j x q_j paired with W2_j x v_j) at bf16 column rate, i.e. 2x
    PE throughput for that slice. The fp8 fraction is the accuracy/speed
    knob: softmax absmax error grows as sqrt(fraction) of ~3.0e-2 (measured
    at fraction 1), and the harness gate is 2e-2. Host pre-scales q/v by 2^4
    and W1/W2 by 2^6 to keep fp8 in the normal range (bf16-path operands get
    the same exact power-of-two scales so the PSUM accumulator is uniform);
    the tanh activation rescales by 2^-10. Stage B and the context matmul
    stay bf16.
  - The partition-dim softmax sum is a DVE add-tree (8 o-blocks -> 1) plus a
    single replicating ones-matmul instead of 8 accumulated ones-matmuls.
  - Per (batch, l-tile of 512): stage A -> tanh -> stage B (8 x 8 accums) ->
    exp -> DVE sum tree -> ones-matmul -> reciprocal -> normalize -> context
    matmul (8 x 8 accums). Softmax+context of step i is emitted after the
    matmul stages of step i+1 so the PE never waits on DVE work.
"""

import numpy as np
import ml_dtypes
from contextlib import ExitStack

import concourse.bass as bass
import concourse.mybir as mybir
import concourse.tile as tile
from concourse import bacc, bass_utils

B, L, H = 32, 1024, 1024
NCORES = 8
BLOC = B // NCORES  # batches per core
P = 128             # partitions
LT = 512            # l-tile (moving free dim)
NLT = L // LT       # l-tiles per batch
NH = H // P         # 128-blocks along h / o / k
NHT = H // LT       # 512-tiles along h (context output)

# Stage-A precision knob: number of 128-blocks (of 8) of the contraction
# whose q AND v streams run in one fp8 DoubleRow matmul; the remaining
# blocks run as two bf16 matmuls each.
NF8 = 3
NR = NH - NF8                 # bf16 remainder blocks
QS = 16.0                     # pre-scale on q/v
WS = 64.0                     # pre-scale on w1/w2
ASCALE = 1.0 / (QS * WS)      # tanh activation rescale

BF16 = mybir.dt.bfloat16
F32 = mybir.dt.float32
F32R = mybir.dt.float32r
FP8 = mybir.dt.float8e4
AFT = mybir.ActivationFunctionType
DR = mybir.MatmulPerfMode.DoubleRow

_PROGRAM_CACHE = {}


def _build_program():
    nc = bacc.Bacc("TRN2", target_bir_lowering=False, debug=False)

    def din(name, shape, dt):
        return nc.dram_tensor(name, shape, dt, kind="ExternalInput").ap()

    ins = {}
    if NF8:
        ins["q8"] = din("q8_in", [BLOC, NF8 * P, L], FP8)
        ins["v8"] = din("v8_in", [BLOC, NF8 * P, L], FP8)
        ins["w18"] = din("w18_in", [NF8 * P, H], FP8)
        ins["w28"] = din("w28_in", [NF8 * P, H], FP8)
    if NF8 < NH:
        ins["qb"] = din("qb_in", [BLOC, NR * P, L], BF16)
        ins["vb16"] = din("vb16_in", [BLOC, NR * P, L], BF16)
        ins["w1b"] = din("w1b_in", [NR * P, H], BF16)
        ins["w2b"] = din("w2b_in", [NR * P, H], BF16)
    ins["vn"] = din("vn_in", [BLOC, L, H], BF16)
    ins["vwt"] = din("vwt_in", [H, H], BF16)
    ins["b12"] = din("b12_in", [P, NH], F32)
    ins["vbt"] = din("vbt_in", [P, NH], F32)
    # float32r ones for the partition-dim softmax sum (memset can't write f32r)
    ins["onesd"] = din("ones_in", [P, P], F32R)

    attT = nc.dram_tensor("att_out", [BLOC, H, L], F32, kind="ExternalOutput").ap()
    ctxo = nc.dram_tensor("ctx_out", [BLOC, L, H], F32, kind="ExternalOutput").ap()

    with tile.TileContext(nc) as tc:
        _kernel_body(tc, ins, attT, ctxo)
    nc.compile()
    return nc


def _kernel_body(tc, ins, attT, ctxo):
    nc = tc.nc
    with ExitStack() as ctx:
        consts = ctx.enter_context(tc.tile_pool(name="consts", bufs=1))
        qpool = ctx.enter_context(tc.tile_pool(name="qpool", bufs=2))
        hpool = ctx.enter_context(tc.tile_pool(name="hpool", bufs=2))
        epool = ctx.enter_context(tc.tile_pool(name="epool", bufs=2))
        apool = ctx.enter_context(tc.tile_pool(name="apool", bufs=2))
        vpool = ctx.enter_context(tc.tile_pool(name="vpool", bufs=2))
        mpool = ctx.enter_context(tc.tile_pool(name="mpool", bufs=2))
        cpool = ctx.enter_context(tc.tile_pool(name="cpool", bufs=3))
        psA = ctx.enter_context(tc.tile_pool(name="psA", bufs=2, space="PSUM"))
        psB = ctx.enter_context(tc.tile_pool(name="psB", bufs=2, space="PSUM"))
        psS = ctx.enter_context(tc.tile_pool(name="psS", bufs=1, space="PSUM"))
        psC = ctx.enter_context(tc.tile_pool(name="psC", bufs=3, space="PSUM"))

        def load_qv(b, lt):
            """Allocate + chunk-DMA the transposed q/v slabs for one l-tile.
            qv8 interleaves (q_j, v_j) per fp8 block for DoubleRow pairing."""
            lsl = slice(lt * LT, (lt + 1) * LT)
            qv8 = qpool.tile([P, NF8, 2, LT], FP8, tag="qv8", name="qv8") \
                if NF8 else None
            qsb = qpool.tile([P, NR, LT], BF16, tag="qsb", name="qsb") \
                if NR else None
            vsb = qpool.tile([P, NR, LT], BF16, tag="vsb", name="vsb") \
                if NR else None
            for j in range(NF8):
                rsl = slice(j * P, (j + 1) * P)
                nc.sync.dma_start(qv8[:, j, 0, :], ins["q8"][b, rsl, lsl])
                nc.sync.dma_start(qv8[:, j, 1, :], ins["v8"][b, rsl, lsl])
            for j in range(0, NR, 2):
                j2 = min(j + 2, NR)
                rsl = slice(j * P, j2 * P)
                nc.sync.dma_start(
                    qsb[:, j:j2, :],
                    ins["qb"][b, rsl, lsl].rearrange("(nh p) l -> p nh l", p=P))
                nc.sync.dma_start(
                    vsb[:, j:j2, :],
                    ins["vb16"][b, rsl, lsl].rearrange("(nh p) l -> p nh l", p=P))
            return qv8, qsb, vsb

        # Resident stage-A weights, contraction 128-block on partitions; fp8
        # (w1_j, w2_j) interleaved pairs first, bf16 remainder after. Chunk
        # loads are interleaved with step 0's q/v chunks in consumption order
        # so the first matmul gates on ~0.5 MB.
        b12s = consts.tile([P, NH], F32)
        nc.sync.dma_start(b12s, ins["b12"])
        vbs = consts.tile([P, NH], F32)
        nc.sync.dma_start(vbs, ins["vbt"])
        ones = consts.tile([P, P], F32R)
        nc.sync.dma_start(ones, ins["onesd"])
        w128 = consts.tile([P, NF8, 2, H], FP8, name="w128") if NF8 else None
        w1sb = consts.tile([P, NR, H], BF16, name="w1sb") if NR else None
        w2sb = consts.tile([P, NR, H], BF16, name="w2sb") if NR else None
        qv0 = qpool.tile([P, NF8, 2, LT], FP8, tag="qv8", name="qv8") \
            if NF8 else None
        qb0 = qpool.tile([P, NR, LT], BF16, tag="qsb", name="qsb") if NR else None
        vb0 = qpool.tile([P, NR, LT], BF16, tag="vsb", name="vsb") if NR else None
        for j in range(NF8):
            rsl = slice(j * P, (j + 1) * P)
            nc.sync.dma_start(w128[:, j, 0, 0:P], ins["w18"][rsl, 0:P])
            nc.sync.dma_start(w128[:, j, 1, 0:P], ins["w28"][rsl, 0:P])
            nc.sync.dma_start(qv0[:, j, 0, :], ins["q8"][0, rsl, 0:LT])
            nc.sync.dma_start(qv0[:, j, 1, :], ins["v8"][0, rsl, 0:LT])
        for j in range(NF8):
            rsl = slice(j * P, (j + 1) * P)
            nc.sync.dma_start(w128[:, j, 0, P:H], ins["w18"][rsl, P:H])
            nc.sync.dma_start(w128[:, j, 1, P:H], ins["w28"][rsl, P:H])
        for j in range(0, NR, 2):
            j2 = min(j + 2, NR)
            rsl = slice(j * P, j2 * P)
            nc.sync.dma_start(w1sb[:, j:j2, :],
                              ins["w1b"][rsl, :].rearrange("(nh p) o -> p nh o", p=P))
            nc.sync.dma_start(w2sb[:, j:j2, :],
                              ins["w2b"][rsl, :].rearrange("(nh p) o -> p nh o", p=P))
            nc.sync.dma_start(
                qb0[:, j:j2, :],
                ins["qb"][0, rsl, 0:LT].rearrange("(nh p) l -> p nh l", p=P))
            nc.sync.dma_start(
                vb0[:, j:j2, :],
                ins["vb16"][0, rsl, 0:LT].rearrange("(nh p) l -> p nh l", p=P))
        vws = consts.tile([P, NH, H], BF16)
        vws_loaded = []

        steps = [(b, lt) for b in range(BLOC) for lt in range(NLT)]
        vnat_tiles = {}

        def emit_stage_a(b, lt, preloaded=None):
            qv8, qsb, vsb = preloaded if preloaded is not None else load_qv(b, lt)

            # Stage A: hT[o, l] = tanh((W1' q'^T + W2' v'^T) * 2^-10 + b1 + b2)
            # fp8 DoubleRow blocks contract q_j AND v_j in one matmul.
            hT = hpool.tile([P, NH, LT], BF16, tag="hT")
            for o in range(NH):
                osl = slice(o * P, (o + 1) * P)
                pa = psA.tile([P, LT], F32, tag="pa")
                for j in range(NF8):
                    nc.tensor.matmul(pa, w128[:, j, :, osl], qv8[:, j, :, :],
                                     start=(j == 0), stop=(NR == 0 and j == NF8 - 1),
                                     perf_mode=DR)
                for j in range(NR):
                    nc.tensor.matmul(pa, w1sb[:, j, osl], qsb[:, j, :],
                                     start=(NF8 == 0 and j == 0), stop=False)
                    nc.tensor.matmul(pa, w2sb[:, j, osl], vsb[:, j, :],
                                     start=False, stop=(j == NR - 1))
                nc.scalar.activation(hT[:, o, :], pa, AFT.Tanh,
                                     bias=b12s[:, o:o + 1], scale=ASCALE)

            if not vws_loaded:
                for ht in range(NH):
                    nc.sync.dma_start(vws[:, ht, :],
                                      ins["vwt"][ht * P:(ht + 1) * P, :])
                vws_loaded.append(True)

            # value in natural [k, h] layout for the context matmul (used ~a full
            # step later, so the DMA is emitted after stage A's)
            if b not in vnat_tiles:
                vnat = vpool.tile([P, NH, H], BF16, tag="vnat")
                for j in range(0, NH, 2):
                    rsl = slice(j * P, (j + 2) * P)
                    nc.sync.dma_start(
                        vnat[:, j:j + 2, :],
                        ins["vn"][b, rsl, :].rearrange("(nk p) h -> p nk h", p=P))
                vnat_tiles.clear()
                vnat_tiles[b] = vnat
            vnat = vnat_tiles[b]
            return hT, vnat

        def emit_stage_b(b, lt, apart, last=False):
            hT, vnat = apart
            # Stage B: expT[o, l] = exp(Vw h + vb)  (no max-subtraction; scores
            # are small).
            # final tile: expT is f32r so the softmax sum can accumulate on
            # the PE right behind each exp, keeping the tail's DVE chain short
            expT = epool.tile([P, NH, LT], F32R if last else F32, tag="expT")
            ps = psS.tile([P, LT], F32, tag="ps")
            for o in range(NH):
                osl = slice(o * P, (o + 1) * P)
                pb = psB.tile([P, LT], F32, tag="pb")
                for ht in range(NH):
                    nc.tensor.matmul(pb, vws[:, ht, osl], hT[:, ht, :],
                                     start=(ht == 0), stop=(ht == NH - 1))
                nc.scalar.activation(expT[:, o, :], pb, AFT.Exp,
                                     bias=vbs[:, o:o + 1], scale=1.0)
                if last:
                    nc.tensor.matmul(ps, ones[:], expT[:, o, :],
                                     start=(o == 0), stop=(o == NH - 1))
            if not last:
                # Partition-dim softmax sums, replicated to all partitions:
                # DVE tree-sum over the 8 o-blocks, then one ones-matmul to
                # replicate the 128 partial sums across partitions.
                t4 = mpool.tile([P, 4, LT], F32, tag="t4")
                t2 = mpool.tile([P, 2, LT], F32, tag="t2")
                s1 = mpool.tile([P, LT], F32R, tag="s1")
                nc.vector.tensor_add(t4, expT[:, 0:4, :], expT[:, 4:8, :])
                nc.vector.tensor_add(t2, t4[:, 0:2, :], t4[:, 2:4, :])
                nc.vector.tensor_add(s1, t2[:, 0, :], t2[:, 1, :])
                nc.tensor.matmul(ps, ones[:], s1[:], start=True, stop=True)
            return (b, lt, expT, ps, vnat)

        def emit_softmax_context(state, last=False):
            b, lt, expT, ps, vnat = state
            lsl = slice(lt * LT, (lt + 1) * LT)
            # last tile's expT is f32r-typed; read through an f32 view, write
            # in-place through the f32r-typed AP (BIR writers-rounded rule)
            expf = expT[:].bitcast(F32) if last else expT
            recip = mpool.tile([P, LT], F32, tag="recip")
            rscr = mpool.tile([P, LT], F32, tag="rscr")
            # ~2 ULP, ~2.8x faster than reciprocal(); sums are ~1e3 so no edge cases
            nc.vector.reciprocal_approx_accurate(recip, ps, rscr)
            attw = apool.tile([P, NH, LT], BF16, tag="attw")
            # all bf16 attw muls first: they gate the context matmuls on PE.
            # On the last step (nothing left to hide the DVE chain behind) do
            # them in l-halves so the first context groups start sooner.
            halves = [slice(0, LT // 2), slice(LT // 2, LT)] if last \
                else [slice(0, LT)]
            for hsl2 in halves:
                for o in range(NH):
                    nc.vector.tensor_mul(attw[:, o, hsl2], expf[:, o, hsl2],
                                         recip[:, hsl2])
            for o in range(NH):
                nc.vector.tensor_mul(expT[:, o, :], expf[:, o, :], recip)
            nc.sync.dma_start(
                attT[b, :, lsl].rearrange("(nh p) l -> p nh l", p=P), expf)

            # Context: ctx[l, h] = sum_k att[k, l] * v[k, h]
            for lb in range(LT // P):
                row0 = lt * LT + lb * P
                for hti in range(NHT):
                    hsl = slice(hti * LT, (hti + 1) * LT)
                    pc = psC.tile([P, LT], F32, tag="pc")
                    for kt in range(NH):
                        nc.tensor.matmul(pc, attw[:, kt, lb * P:(lb + 1) * P],
                                         vnat[:, kt, hsl],
                                         start=(kt == 0), stop=(kt == NH - 1))
                    cs = cpool.tile([P, LT], F32, tag="cs")
                    # PSUM->SBUF evacuation alternating ScalarE/DVE so neither
                    # queue's backlog blocks psC slot reuse for long
                    if hti == 0:
                        nc.scalar.activation(cs, pc, AFT.Copy)
                    else:
                        nc.vector.tensor_copy(cs, pc)
                    nc.sync.dma_start(ctxo[b, row0:row0 + P, hsl], cs)

        pending = None
        nsteps = len(steps)
        for i, (b, lt) in enumerate(steps):
            apart = emit_stage_a(
                b, lt, preloaded=(qv0, qb0, vb0) if i == 0 else None)
            state = emit_stage_b(b, lt, apart, last=(i == nsteps - 1))
            if pending is not None:
                emit_softmax_context(pending)
            pending = state
        emit_softmax_context(pending, last=True)


def _get_program():
    if "nc" not in _PROGRAM_CACHE:
        _PROGRAM_CACHE["nc"] = _build_program()
    return _PROGRAM_CACHE["nc"]


def _prep_in_maps(query, value, w1_w, w1_b, w2_w, w2_b, v_w, v_b):
    bf16 = ml_dtypes.bfloat16
    fp8 = ml_dtypes.float8_e4m3
    # [h, o] layouts; fp8 rows scaled by WS, bf16 rows too (exact power of
    # two) so the PSUM accumulator has one uniform scale.
    w1t = np.ascontiguousarray(w1_w.T) * WS
    w2t = np.ascontiguousarray(w2_w.T) * WS
    vwt = v_w.T.astype(bf16)
    b12 = np.ascontiguousarray((w1_b + w2_b).astype(np.float32).reshape(NH, P).T)
    vbt = np.ascontiguousarray(v_b.astype(np.float32).reshape(NH, P).T)

    base = {
        "vwt_in": vwt,
        "b12_in": b12,
        "vbt_in": vbt,
        "ones_in": np.ones((P, P), np.float32),
    }
    if NF8:
        base["w18_in"] = w1t[:NF8 * P].astype(fp8)
        base["w28_in"] = w2t[:NF8 * P].astype(fp8)
    if NR:
        base["w1b_in"] = w1t[NF8 * P:].astype(bf16)
        base["w2b_in"] = w2t[NF8 * P:].astype(bf16)

    in_maps = []
    for c in range(NCORES):
        sl = slice(c * BLOC, (c + 1) * BLOC)
        qT = query[sl].transpose(0, 2, 1) * QS
        vT = value[sl].transpose(0, 2, 1) * QS
        m = dict(base)
        if NF8:
            m["q8_in"] = qT[:, :NF8 * P].astype(fp8)
            m["v8_in"] = vT[:, :NF8 * P].astype(fp8)
        if NR:
            m["qb_in"] = qT[:, NF8 * P:].astype(bf16)
            m["vb16_in"] = vT[:, NF8 * P:].astype(bf16)
        m["vn_in"] = value[sl].astype(bf16)
        in_maps.append(m)
    return in_maps


def run_sharded(inputs, **run_kwargs):
    """Build in_maps, run on 8 cores, return (att, ctx, BassKernelResults)."""
    query = np.asarray(inputs["query"], dtype=np.float32)
    value = np.asarray(inputs["value"], dtype=np.float32)
    in_maps = _prep_in_maps(
        query, value,
        np.asarray(inputs["w1_w"], np.float32), np.asarray(inputs["w1_b"], np.float32),
        np.asarray(inputs["w2_w"], np.float32), np.asarray(inputs["w2_b"], np.float32),
        np.asarray(inputs["v_w"], np.float32), np.asarray(inputs["v_b"], np.float32),
    )
    nc = _get_program()
    res = bass_utils.run_bass_kernel_spmd(
        nc, in_maps, core_ids=list(range(NCORES)), **run_kwargs)

    att = np.empty((B, L, H), np.float32)
    ctxv = np.empty((B, L, H), np.float32)
    for c in range(NCORES):
        sl = slice(c * BLOC, (c + 1) * BLOC)
        att[sl] = res.results[c]["att_out"].transpose(0, 2, 1)
        ctxv[sl] = res.results[c]["ctx_out"]
    return att, ctxv, res


def kernel(**inputs):
    att, ctxv, _ = run_sharded(inputs)
    return att, ctxv


# revision 14
# speedup vs baseline: 1.0067x; 1.0067x over previous
"""Bahdanau attention kernel for Trainium2, 8-core data-parallel.

Problem (B=32, L=1024, H=1024, fp32):
    h     = tanh(q @ W1.T + b1 + v @ W2.T + b2)        # (B, L, H)
    score = h @ Vw.T + vb                              # (B, L, H)
    att   = softmax(score, axis=-1)                    # (B, L, H)
    ctx   = att @ v                                    # (B, L, H)  (bmm over kv dim)
    returns (att, ctx)

Strategy:
  - Data-parallel: 4 batches per core on 8 cores.
  - Everything on-device runs in a TRANSPOSED layout [h, l] so that the
    contraction dim (h / k) always lands on SBUF partitions and no on-device
    transposes are needed. The host pre-transposes q and value per batch
    (and pre-transposes the weight matrices), and transposes the attention
    weights output back after gathering.
  - A tunable slice of stage A (the q@W1 + v@W2 preactivation) runs in fp8e4
    with perf_mode=DoubleRow: one matmul contracts the j-th 128-block of BOTH
    streams (W1_j x q_j paired with W2_j x v_j) at bf16 column rate, i.e. 2x
    PE throughput for that slice. The fp8 fraction is the accuracy/speed
    knob: softmax absmax error grows as sqrt(fraction) of ~3.0e-2 (measured
    at fraction 1), and the harness gate is 2e-2. Host pre-scales q/v by 2^4
    and W1/W2 by 2^6 to keep fp8 in the normal range (bf16-path operands get
    the same exact power-of-two scales so the PSUM accumulator is uniform);
    the tanh activation rescales by 2^-10. Stage B and the context matmul
    stay bf16.
  - The partition-dim softmax sum is a DVE add-tree (8 o-blocks -> 1) plus a
    single replicating ones-matmul instead of 8 accumulated ones-matmuls.
  - Per (batch, l-tile of 512): stage A -> tanh -> stage B (8 x 8 accums) ->
    exp -> DVE sum tree -> ones-matmul -> reciprocal -> normalize -> context
    matmul (8 x 8 accums). Softmax+context of step i is emitted after the
    matmul stages of step i+1 so the PE never waits on DVE work.
"""

import numpy as np
import ml_dtypes
from contextlib import ExitStack

import concourse.bass as bass
import concourse.mybir as mybir
import concourse.tile as tile
from concourse import bacc, bass_utils

B, L, H = 32, 1024, 1024
NCORES = 8
BLOC = B // NCORES  # batches per core
P = 128             # partitions
LT = 512            # l-tile (moving free dim)
NLT = L // LT       # l-tiles per batch
NH = H // P         # 128-blocks along h / o / k
NHT = H // LT       # 512-tiles along h (context output)

# Stage-A precision knob: number of 128-blocks (of 8) of the contraction
# whose q AND v streams run in one fp8 DoubleRow matmul; the remaining
# blocks run as two bf16 matmuls each.
NF8 = 3
NR = NH - NF8                 # bf16 remainder blocks
QS = 16.0                     # pre-scale on q/v
WS = 64.0                     # pre-scale on w1/w2
ASCALE = 1.0 / (QS * WS)      # tanh activation rescale

BF16 = mybir.dt.bfloat16
F32 = mybir.dt.float32
F32R = mybir.dt.float32r
FP8 = mybir.dt.float8e4
AFT = mybir.ActivationFunctionType
DR = mybir.MatmulPerfMode.DoubleRow

_PROGRAM_CACHE = {}


def _build_program():
    nc = bacc.Bacc("TRN2", target_bir_lowering=False, debug=False)

    def din(name, shape, dt):
        return nc.dram_tensor(name, shape, dt, kind="ExternalInput").ap()

    ins = {}
    if NF8:
        ins["q8"] = din("q8_in", [BLOC, NF8 * P, L], FP8)
        ins["v8"] = din("v8_in", [BLOC, NF8 * P, L], FP8)
        ins["w18"] = din("w18_in", [NF8 * P, H], FP8)
        ins["w28"] = din("w28_in", [NF8 * P, H], FP8)
    if NF8 < NH:
        ins["qb"] = din("qb_in", [BLOC, NR * P, L], BF16)
        ins["vb16"] = din("vb16_in", [BLOC, NR * P, L], BF16)
        ins["w1b"] = din("w1b_in", [NR * P, H], BF16)
        ins["w2b"] = din("w2b_in", [NR * P, H], BF16)
    ins["vn"] = din("vn_in", [BLOC, L, H], BF16)
    ins["vwt"] = din("vwt_in", [H, H], BF16)
    ins["b12"] = din("b12_in", [P, NH], F32)
    ins["vbt"] = din("vbt_in", [P, NH], F32)
    # float32r ones for the partition-dim softmax sum (memset can't write f32r)
    ins["onesd"] = din("ones_in", [P, P], F32R)

    attT = nc.dram_tensor("att_out", [BLOC, H, L], F32, kind="ExternalOutput").ap()
    ctxo = nc.dram_tensor("ctx_out", [BLOC, L, H], F32, kind="ExternalOutput").ap()

    with tile.TileContext(nc) as tc:
        _kernel_body(tc, ins, attT, ctxo)
    nc.compile()
    return nc


def _kernel_body(tc, ins, attT, ctxo):
    nc = tc.nc
    with ExitStack() as ctx:
        consts = ctx.enter_context(tc.tile_pool(name="consts", bufs=1))
        qpool = ctx.enter_context(tc.tile_pool(name="qpool", bufs=2))
        hpool = ctx.enter_context(tc.tile_pool(name="hpool", bufs=2))
        epool = ctx.enter_context(tc.tile_pool(name="epool", bufs=2))
        apool = ctx.enter_context(tc.tile_pool(name="apool", bufs=2))
        vpool = ctx.enter_context(tc.tile_pool(name="vpool", bufs=2))
        mpool = ctx.enter_context(tc.tile_pool(name="mpool", bufs=2))
        cpool = ctx.enter_context(tc.tile_pool(name="cpool", bufs=3))
        psA = ctx.enter_context(tc.tile_pool(name="psA", bufs=2, space="PSUM"))
        psB = ctx.enter_context(tc.tile_pool(name="psB", bufs=2, space="PSUM"))
        psS = ctx.enter_context(tc.tile_pool(name="psS", bufs=1, space="PSUM"))
        psC = ctx.enter_context(tc.tile_pool(name="psC", bufs=3, space="PSUM"))

        def load_qv(b, lt):
            """Allocate + chunk-DMA the transposed q/v slabs for one l-tile.
            qv8 interleaves (q_j, v_j) per fp8 block for DoubleRow pairing."""
            lsl = slice(lt * LT, (lt + 1) * LT)
            qv8 = qpool.tile([P, NF8, 2, LT], FP8, tag="qv8", name="qv8") \
                if NF8 else None
            qsb = qpool.tile([P, NR, LT], BF16, tag="qsb", name="qsb") \
                if NR else None
            vsb = qpool.tile([P, NR, LT], BF16, tag="vsb", name="vsb") \
                if NR else None
            for j in range(NF8):
                rsl = slice(j * P, (j + 1) * P)
                nc.sync.dma_start(qv8[:, j, 0, :], ins["q8"][b, rsl, lsl])
                nc.sync.dma_start(qv8[:, j, 1, :], ins["v8"][b, rsl, lsl])
            for j in range(0, NR, 2):
                j2 = min(j + 2, NR)
                rsl = slice(j * P, j2 * P)
                nc.sync.dma_start(
                    qsb[:, j:j2, :],
                    ins["qb"][b, rsl, lsl].rearrange("(nh p) l -> p nh l", p=P))
                nc.sync.dma_start(
                    vsb[:, j:j2, :],
                    ins["vb16"][b, rsl, lsl].rearrange("(nh p) l -> p nh l", p=P))
            return qv8, qsb, vsb

        # Resident stage-A weights, contraction 128-block on partitions; fp8
        # (w1_j, w2_j) interleaved pairs first, bf16 remainder after. Chunk
        # loads are interleaved with step 0's q/v chunks in consumption order
        # so the first matmul gates on ~0.5 MB.
        b12s = consts.tile([P, NH], F32)
        nc.sync.dma_start(b12s, ins["b12"])
        vbs = consts.tile([P, NH], F32)
        nc.sync.dma_start(vbs, ins["vbt"])
        ones = consts.tile([P, P], F32R)
        nc.sync.dma_start(ones, ins["onesd"])
        w128 = consts.tile([P, NF8, 2, H], FP8, name="w128") if NF8 else None
        w1sb = consts.tile([P, NR, H], BF16, name="w1sb") if NR else None
        w2sb = consts.tile([P, NR, H], BF16, name="w2sb") if NR else None
        qv0 = qpool.tile([P, NF8, 2, LT], FP8, tag="qv8", name="qv8") \
            if NF8 else None
        qb0 = qpool.tile([P, NR, LT], BF16, tag="qsb", name="qsb") if NR else None
        vb0 = qpool.tile([P, NR, LT], BF16, tag="vsb", name="vsb") if NR else None
        for j in range(NF8):
            rsl = slice(j * P, (j + 1) * P)
            nc.sync.dma_start(w128[:, j, 0, :], ins["w18"][rsl, :])
            nc.sync.dma_start(w128[:, j, 1, :], ins["w28"][rsl, :])
            nc.sync.dma_start(qv0[:, j, 0, :], ins["q8"][0, rsl, 0:LT])
            nc.sync.dma_start(qv0[:, j, 1, :], ins["v8"][0, rsl, 0:LT])
        for j in range(0, NR, 2):
            j2 = min(j + 2, NR)
            rsl = slice(j * P, j2 * P)
            nc.sync.dma_start(w1sb[:, j:j2, :],
                              ins["w1b"][rsl, :].rearrange("(nh p) o -> p nh o", p=P))
            nc.sync.dma_start(w2sb[:, j:j2, :],
                              ins["w2b"][rsl, :].rearrange("(nh p) o -> p nh o", p=P))
            nc.sync.dma_start(
                qb0[:, j:j2, :],
                ins["qb"][0, rsl, 0:LT].rearrange("(nh p) l -> p nh l", p=P))
            nc.sync.dma_start(
                vb0[:, j:j2, :],
                ins["vb16"][0, rsl, 0:LT].rearrange("(nh p) l -> p nh l", p=P))
        vws = consts.tile([P, NH, H], BF16)
        vws_loaded = []

        steps = [(b, lt) for b in range(BLOC) for lt in range(NLT)]
        vnat_tiles = {}

        def emit_stage_a(b, lt, preloaded=None):
            qv8, qsb, vsb = preloaded if preloaded is not None else load_qv(b, lt)

            # Stage A: hT[o, l] = tanh((W1' q'^T + W2' v'^T) * 2^-10 + b1 + b2)
            # fp8 DoubleRow blocks contract q_j AND v_j in one matmul.
            hT = hpool.tile([P, NH, LT], BF16, tag="hT")
            for o in range(NH):
                osl = slice(o * P, (o + 1) * P)
                pa = psA.tile([P, LT], F32, tag="pa")
                for j in range(NF8):
                    nc.tensor.matmul(pa, w128[:, j, :, osl], qv8[:, j, :, :],
                                     start=(j == 0), stop=(NR == 0 and j == NF8 - 1),
                                     perf_mode=DR)
                for j in range(NR):
                    nc.tensor.matmul(pa, w1sb[:, j, osl], qsb[:, j, :],
                                     start=(NF8 == 0 and j == 0), stop=False)
                    nc.tensor.matmul(pa, w2sb[:, j, osl], vsb[:, j, :],
                                     start=False, stop=(j == NR - 1))
                nc.scalar.activation(hT[:, o, :], pa, AFT.Tanh,
                                     bias=b12s[:, o:o + 1], scale=ASCALE)

            if not vws_loaded:
                for ht in range(NH):
                    nc.sync.dma_start(vws[:, ht, :],
                                      ins["vwt"][ht * P:(ht + 1) * P, :])
                vws_loaded.append(True)

            # value in natural [k, h] layout for the context matmul (used ~a full
            # step later, so the DMA is emitted after stage A's)
            if b not in vnat_tiles:
                vnat = vpool.tile([P, NH, H], BF16, tag="vnat")
                for j in range(0, NH, 2):
                    rsl = slice(j * P, (j + 2) * P)
                    nc.sync.dma_start(
                        vnat[:, j:j + 2, :],
                        ins["vn"][b, rsl, :].rearrange("(nk p) h -> p nk h", p=P))
                vnat_tiles.clear()
                vnat_tiles[b] = vnat
            vnat = vnat_tiles[b]
            return hT, vnat

        def emit_stage_b(b, lt, apart, last=False):
            hT, vnat = apart
            # Stage B: expT[o, l] = exp(Vw h + vb)  (no max-subtraction; scores
            # are small).
            # final tile: expT is f32r so the softmax sum can accumulate on
            # the PE right behind each exp, keeping the tail's DVE chain short
            expT = epool.tile([P, NH, LT], F32R if last else F32, tag="expT")
            ps = psS.tile([P, LT], F32, tag="ps")
            for o in range(NH):
                osl = slice(o * P, (o + 1) * P)
                pb = psB.tile([P, LT], F32, tag="pb")
                for ht in range(NH):
                    nc.tensor.matmul(pb, vws[:, ht, osl], hT[:, ht, :],
                                     start=(ht == 0), stop=(ht == NH - 1))
                nc.scalar.activation(expT[:, o, :], pb, AFT.Exp,
                                     bias=vbs[:, o:o + 1], scale=1.0)
                if last:
                    nc.tensor.matmul(ps, ones[:], expT[:, o, :],
                                     start=(o == 0), stop=(o == NH - 1))
            if not last:
                # Partition-dim softmax sums, replicated to all partitions:
                # DVE tree-sum over the 8 o-blocks, then one ones-matmul to
                # replicate the 128 partial sums across partitions.
                t4 = mpool.tile([P, 4, LT], F32, tag="t4")
                t2 = mpool.tile([P, 2, LT], F32, tag="t2")
                s1 = mpool.tile([P, LT], F32R, tag="s1")
                nc.vector.tensor_add(t4, expT[:, 0:4, :], expT[:, 4:8, :])
                nc.vector.tensor_add(t2, t4[:, 0:2, :], t4[:, 2:4, :])
                nc.vector.tensor_add(s1, t2[:, 0, :], t2[:, 1, :])
                nc.tensor.matmul(ps, ones[:], s1[:], start=True, stop=True)
            return (b, lt, expT, ps, vnat)

        def emit_softmax_context(state, last=False):
            b, lt, expT, ps, vnat = state
            lsl = slice(lt * LT, (lt + 1) * LT)
            # last tile's expT is f32r-typed; read through an f32 view, write
            # in-place through the f32r-typed AP (BIR writers-rounded rule)
            expf = expT[:].bitcast(F32) if last else expT
            recip = mpool.tile([P, LT], F32, tag="recip")
            rscr = mpool.tile([P, LT], F32, tag="rscr")
            # ~2 ULP, ~2.8x faster than reciprocal(); sums are ~1e3 so no edge cases
            nc.vector.reciprocal_approx_accurate(recip, ps, rscr)
            attw = apool.tile([P, NH, LT], BF16, tag="attw")
            # all bf16 attw muls first: they gate the context matmuls on PE.
            # On the last step (nothing left to hide the DVE chain behind) do
            # them in l-halves so the first context groups start sooner.
            halves = [slice(0, LT // 2), slice(LT // 2, LT)] if last \
                else [slice(0, LT)]
            for hsl2 in halves:
                for o in range(NH):
                    nc.vector.tensor_mul(attw[:, o, hsl2], expf[:, o, hsl2],
                                         recip[:, hsl2])
            for o in range(NH):
                nc.vector.tensor_mul(expT[:, o, :], expf[:, o, :], recip)
            nc.sync.dma_start(
                attT[b, :, lsl].rearrange("(nh p) l -> p nh l", p=P), expf)

            # Context: ctx[l, h] = sum_k att[k, l] * v[k, h]
            for lb in range(LT // P):
                row0 = lt * LT + lb * P
                for hti in range(NHT):
                    hsl = slice(hti * LT, (hti + 1) * LT)
                    pc = psC.tile([P, LT], F32, tag="pc")
                    for kt in range(NH):
                        nc.tensor.matmul(pc, attw[:, kt, lb * P:(lb + 1) * P],
                                         vnat[:, kt, hsl],
                                         start=(kt == 0), stop=(kt == NH - 1))
                    cs = cpool.tile([P, LT], F32, tag="cs")
                    # PSUM->SBUF evacuation alternating ScalarE/DVE so neither
                    # queue's backlog blocks psC slot reuse for long
                    if hti == 0:
                        nc.scalar.activation(cs, pc, AFT.Copy)
                    else:
                        nc.vector.tensor_copy(cs, pc)
                    nc.sync.dma_start(ctxo[b, row0:row0 + P, hsl], cs)

        pending = None
        nsteps = len(steps)
        for i, (b, lt) in enumerate(steps):
            apart = emit_stage_a(
                b, lt, preloaded=(qv0, qb0, vb0) if i == 0 else None)
            state = emit_stage_b(b, lt, apart, last=(i == nsteps - 1))
            if pending is not None:
                emit_softmax_context(pending)
            pending = state
        emit_softmax_context(pending, last=True)


def _get_program():
    if "nc" not in _PROGRAM_CACHE:
        _PROGRAM_CACHE["nc"] = _build_program()
    return _PROGRAM_CACHE["nc"]


def _prep_in_maps(query, value, w1_w, w1_b, w2_w, w2_b, v_w, v_b):
    bf16 = ml_dtypes.bfloat16
    fp8 = ml_dtypes.float8_e4m3
    # [h, o] layouts; fp8 rows scaled by WS, bf16 rows too (exact power of
    # two) so the PSUM accumulator has one uniform scale.
    w1t = np.ascontiguousarray(w1_w.T) * WS
    w2t = np.ascontiguousarray(w2_w.T) * WS
    vwt = v_w.T.astype(bf16)
    b12 = np.ascontiguousarray((w1_b + w2_b).astype(np.float32).reshape(NH, P).T)
    vbt = np.ascontiguousarray(v_b.astype(np.float32).reshape(NH, P).T)

    base = {
        "vwt_in": vwt,
        "b12_in": b12,
        "vbt_in": vbt,
        "ones_in": np.ones((P, P), np.float32),
    }
    if NF8:
        base["w18_in"] = w1t[:NF8 * P].astype(fp8)
        base["w28_in"] = w2t[:NF8 * P].astype(fp8)
    if NR:
        base["w1b_in"] = w1t[NF8 * P:].astype(bf16)
        base["w2b_in"] = w2t[NF8 * P:].astype(bf16)

    in_maps = []
    for c in range(NCORES):
        sl = slice(c * BLOC, (c + 1) * BLOC)
        qT = query[sl].transpose(0, 2, 1) * QS
        vT = value[sl].transpose(0, 2, 1) * QS
        m = dict(base)
        if NF8:
            m["q8_in"] = qT[:, :NF8 * P].astype(fp8)
            m["v8_in"] = vT[:, :NF8 * P].astype(fp8)
        if NR:
            m["qb_in"] = qT[:, NF8 * P:].astype(bf16)
            m["vb16_in"] = vT[:, NF8 * P:].astype(bf16)
        m["vn_in"] = value[sl].astype(bf16)
        in_maps.append(m)
    return in_maps


def run_sharded(inputs, **run_kwargs):
    """Build in_maps, run on 8 cores, return (att, ctx, BassKernelResults)."""
    query = np.asarray(inputs["query"], dtype=np.float32)
    value = np.asarray(inputs["value"], dtype=np.float32)
    in_maps = _prep_in_maps(
        query, value,
        np.asarray(inputs["w1_w"], np.float32), np.asarray(inputs["w1_b"], np.float32),
        np.asarray(inputs["w2_w"], np.float32), np.asarray(inputs["w2_b"], np.float32),
        np.asarray(inputs["v_w"], np.float32), np.asarray(inputs["v_b"], np.float32),
    )
    nc = _get_program()
    res = bass_utils.run_bass_kernel_spmd(
        nc, in_maps, core_ids=list(range(NCORES)), **run_kwargs)

    att = np.empty((B, L, H), np.float32)
    ctxv = np.empty((B, L, H), np.float32)
    for c in range(NCORES):
        sl = slice(c * BLOC, (c + 1) * BLOC)
        att[sl] = res.results[c]["att_out"].transpose(0, 2, 1)
        ctxv[sl] = res.results[c]["ctx_out"]
    return att, ctxv, res


def kernel(**inputs):
    att, ctxv, _ = run_sharded(inputs)
    return att, ctxv


# revision 15
# speedup vs baseline: 1.0071x; 1.0005x over previous
"""Bahdanau attention kernel for Trainium2, 8-core data-parallel.

Problem (B=32, L=1024, H=1024, fp32):
    h     = tanh(q @ W1.T + b1 + v @ W2.T + b2)        # (B, L, H)
    score = h @ Vw.T + vb                              # (B, L, H)
    att   = softmax(score, axis=-1)                    # (B, L, H)
    ctx   = att @ v                                    # (B, L, H)  (bmm over kv dim)
    returns (att, ctx)

Strategy:
  - Data-parallel: 4 batches per core on 8 cores.
  - Everything on-device runs in a TRANSPOSED layout [h, l] so that the
    contraction dim (h / k) always lands on SBUF partitions and no on-device
    transposes are needed. The host pre-transposes q and value per batch
    (and pre-transposes the weight matrices), and transposes the attention
    weights output back after gathering.
  - A tunable slice of stage A (the q@W1 + v@W2 preactivation) runs in fp8e4
    with perf_mode=DoubleRow: one matmul contracts the j-th 128-block of BOTH
    streams (W1_j x q_j paired with W2_j x v_j) at bf16 column rate, i.e. 2x
    PE throughput for that slice. The fp8 fraction is the accuracy/speed
    knob: softmax absmax error grows as sqrt(fraction) of ~3.0e-2 (measured
    at fraction 1), and the harness gate is 2e-2. Host pre-scales q/v by 2^4
    and W1/W2 by 2^6 to keep fp8 in the normal range (bf16-path operands get
    the same exact power-of-two scales so the PSUM accumulator is uniform);
    the tanh activation rescales by 2^-10. Stage B and the context matmul
    stay bf16.
  - The partition-dim softmax sum is a DVE add-tree (8 o-blocks -> 1) plus a
    single replicating ones-matmul instead of 8 accumulated ones-matmuls.
  - Per (batch, l-tile of 512): stage A -> tanh -> stage B (8 x 8 accums) ->
    exp -> DVE sum tree -> ones-matmul -> reciprocal -> normalize -> context
    matmul (8 x 8 accums). Softmax+context of step i is emitted after the
    matmul stages of step i+1 so the PE never waits on DVE work.
"""

import numpy as np
import ml_dtypes
from contextlib import ExitStack

import concourse.bass as bass
import concourse.mybir as mybir
import concourse.tile as tile
from concourse import bacc, bass_utils

B, L, H = 32, 1024, 1024
NCORES = 8
BLOC = B // NCORES  # batches per core
P = 128             # partitions
LT = 512            # l-tile (moving free dim)
NLT = L // LT       # l-tiles per batch
NH = H // P         # 128-blocks along h / o / k
NHT = H // LT       # 512-tiles along h (context output)

# Stage-A precision knob: number of 128-blocks (of 8) of the contraction
# whose q AND v streams run in one fp8 DoubleRow matmul; the remaining
# blocks run as two bf16 matmuls each.
NF8 = 3
NR = NH - NF8                 # bf16 remainder blocks
QS = 16.0                     # pre-scale on q/v
WS = 64.0                     # pre-scale on w1/w2
ASCALE = 1.0 / (QS * WS)      # tanh activation rescale

BF16 = mybir.dt.bfloat16
F32 = mybir.dt.float32
F32R = mybir.dt.float32r
FP8 = mybir.dt.float8e4
AFT = mybir.ActivationFunctionType
DR = mybir.MatmulPerfMode.DoubleRow

_PROGRAM_CACHE = {}


def _build_program():
    nc = bacc.Bacc("TRN2", target_bir_lowering=False, debug=False)

    def din(name, shape, dt):
        return nc.dram_tensor(name, shape, dt, kind="ExternalInput").ap()

    ins = {}
    if NF8:
        ins["q8"] = din("q8_in", [BLOC, NF8 * P, L], FP8)
        ins["v8"] = din("v8_in", [BLOC, NF8 * P, L], FP8)
        ins["w18"] = din("w18_in", [NF8 * P, H], FP8)
        ins["w28"] = din("w28_in", [NF8 * P, H], FP8)
    if NF8 < NH:
        ins["qb"] = din("qb_in", [BLOC, NR * P, L], BF16)
        ins["vb16"] = din("vb16_in", [BLOC, NR * P, L], BF16)
        ins["w1b"] = din("w1b_in", [NR * P, H], BF16)
        ins["w2b"] = din("w2b_in", [NR * P, H], BF16)
    ins["vn"] = din("vn_in", [BLOC, L, H], BF16)
    ins["vwt"] = din("vwt_in", [H, H], BF16)
    ins["b12"] = din("b12_in", [P, NH], F32)
    ins["vbt"] = din("vbt_in", [P, NH], F32)
    # float32r ones for the partition-dim softmax sum (memset can't write f32r)
    ins["onesd"] = din("ones_in", [P, P], F32R)

    attT = nc.dram_tensor("att_out", [BLOC, H, L], F32, kind="ExternalOutput").ap()
    # bf16 context output (host upcasts): halves the ctx SBUF-read + HBM-write
    # DMA traffic; ctx error budget has ~2x headroom vs the att one
    ctxo = nc.dram_tensor("ctx_out", [BLOC, L, H], BF16, kind="ExternalOutput").ap()

    with tile.TileContext(nc) as tc:
        _kernel_body(tc, ins, attT, ctxo)
    nc.compile()
    return nc


def _kernel_body(tc, ins, attT, ctxo):
    nc = tc.nc
    with ExitStack() as ctx:
        consts = ctx.enter_context(tc.tile_pool(name="consts", bufs=1))
        qpool = ctx.enter_context(tc.tile_pool(name="qpool", bufs=2))
        hpool = ctx.enter_context(tc.tile_pool(name="hpool", bufs=2))
        epool = ctx.enter_context(tc.tile_pool(name="epool", bufs=2))
        apool = ctx.enter_context(tc.tile_pool(name="apool", bufs=2))
        vpool = ctx.enter_context(tc.tile_pool(name="vpool", bufs=2))
        mpool = ctx.enter_context(tc.tile_pool(name="mpool", bufs=2))
        cpool = ctx.enter_context(tc.tile_pool(name="cpool", bufs=3))
        psA = ctx.enter_context(tc.tile_pool(name="psA", bufs=2, space="PSUM"))
        psB = ctx.enter_context(tc.tile_pool(name="psB", bufs=2, space="PSUM"))
        psS = ctx.enter_context(tc.tile_pool(name="psS", bufs=1, space="PSUM"))
        psC = ctx.enter_context(tc.tile_pool(name="psC", bufs=3, space="PSUM"))

        def load_qv(b, lt):
            """Allocate + chunk-DMA the transposed q/v slabs for one l-tile.
            qv8 interleaves (q_j, v_j) per fp8 block for DoubleRow pairing."""
            lsl = slice(lt * LT, (lt + 1) * LT)
            qv8 = qpool.tile([P, NF8, 2, LT], FP8, tag="qv8", name="qv8") \
                if NF8 else None
            qsb = qpool.tile([P, NR, LT], BF16, tag="qsb", name="qsb") \
                if NR else None
            vsb = qpool.tile([P, NR, LT], BF16, tag="vsb", name="vsb") \
                if NR else None
            for j in range(NF8):
                rsl = slice(j * P, (j + 1) * P)
                nc.sync.dma_start(qv8[:, j, 0, :], ins["q8"][b, rsl, lsl])
                nc.sync.dma_start(qv8[:, j, 1, :], ins["v8"][b, rsl, lsl])
            for j in range(0, NR, 2):
                j2 = min(j + 2, NR)
                rsl = slice(j * P, j2 * P)
                nc.sync.dma_start(
                    qsb[:, j:j2, :],
                    ins["qb"][b, rsl, lsl].rearrange("(nh p) l -> p nh l", p=P))
                nc.sync.dma_start(
                    vsb[:, j:j2, :],
                    ins["vb16"][b, rsl, lsl].rearrange("(nh p) l -> p nh l", p=P))
            return qv8, qsb, vsb

        # Resident stage-A weights, contraction 128-block on partitions; fp8
        # (w1_j, w2_j) interleaved pairs first, bf16 remainder after. Chunk
        # loads are interleaved with step 0's q/v chunks in consumption order
        # so the first matmul gates on ~0.5 MB.
        b12s = consts.tile([P, NH], F32)
        nc.sync.dma_start(b12s, ins["b12"])
        vbs = consts.tile([P, NH], F32)
        nc.sync.dma_start(vbs, ins["vbt"])
        ones = consts.tile([P, P], F32R)
        nc.sync.dma_start(ones, ins["onesd"])
        w128 = consts.tile([P, NF8, 2, H], FP8, name="w128") if NF8 else None
        w1sb = consts.tile([P, NR, H], BF16, name="w1sb") if NR else None
        w2sb = consts.tile([P, NR, H], BF16, name="w2sb") if NR else None
        qv0 = qpool.tile([P, NF8, 2, LT], FP8, tag="qv8", name="qv8") \
            if NF8 else None
        qb0 = qpool.tile([P, NR, LT], BF16, tag="qsb", name="qsb") if NR else None
        vb0 = qpool.tile([P, NR, LT], BF16, tag="vsb", name="vsb") if NR else None
        for j in range(NF8):
            rsl = slice(j * P, (j + 1) * P)
            nc.sync.dma_start(w128[:, j, 0, :], ins["w18"][rsl, :])
            nc.sync.dma_start(w128[:, j, 1, :], ins["w28"][rsl, :])
            nc.sync.dma_start(qv0[:, j, 0, :], ins["q8"][0, rsl, 0:LT])
            nc.sync.dma_start(qv0[:, j, 1, :], ins["v8"][0, rsl, 0:LT])
        for j in range(0, NR, 2):
            j2 = min(j + 2, NR)
            rsl = slice(j * P, j2 * P)
            nc.sync.dma_start(w1sb[:, j:j2, :],
                              ins["w1b"][rsl, :].rearrange("(nh p) o -> p nh o", p=P))
            nc.sync.dma_start(w2sb[:, j:j2, :],
                              ins["w2b"][rsl, :].rearrange("(nh p) o -> p nh o", p=P))
            nc.sync.dma_start(
                qb0[:, j:j2, :],
                ins["qb"][0, rsl, 0:LT].rearrange("(nh p) l -> p nh l", p=P))
            nc.sync.dma_start(
                vb0[:, j:j2, :],
                ins["vb16"][0, rsl, 0:LT].rearrange("(nh p) l -> p nh l", p=P))
        vws = consts.tile([P, NH, H], BF16)
        vws_loaded = []

        steps = [(b, lt) for b in range(BLOC) for lt in range(NLT)]
        vnat_tiles = {}

        def emit_stage_a(b, lt, preloaded=None):
            qv8, qsb, vsb = preloaded if preloaded is not None else load_qv(b, lt)

            # Stage A: hT[o, l] = tanh((W1' q'^T + W2' v'^T) * 2^-10 + b1 + b2)
            # fp8 DoubleRow blocks contract q_j AND v_j in one matmul.
            hT = hpool.tile([P, NH, LT], BF16, tag="hT")
            for o in range(NH):
                osl = slice(o * P, (o + 1) * P)
                pa = psA.tile([P, LT], F32, tag="pa")
                for j in range(NF8):
                    nc.tensor.matmul(pa, w128[:, j, :, osl], qv8[:, j, :, :],
                                     start=(j == 0), stop=(NR == 0 and j == NF8 - 1),
                                     perf_mode=DR)
                for j in range(NR):
                    nc.tensor.matmul(pa, w1sb[:, j, osl], qsb[:, j, :],
                                     start=(NF8 == 0 and j == 0), stop=False)
                    nc.tensor.matmul(pa, w2sb[:, j, osl], vsb[:, j, :],
                                     start=False, stop=(j == NR - 1))
                nc.scalar.activation(hT[:, o, :], pa, AFT.Tanh,
                                     bias=b12s[:, o:o + 1], scale=ASCALE)

            if not vws_loaded:
                for ht in range(NH):
                    nc.sync.dma_start(vws[:, ht, :],
                                      ins["vwt"][ht * P:(ht + 1) * P, :])
                vws_loaded.append(True)

            # value in natural [k, h] layout for the context matmul (used ~a full
            # step later, so the DMA is emitted after stage A's)
            if b not in vnat_tiles:
                vnat = vpool.tile([P, NH, H], BF16, tag="vnat")
                for j in range(0, NH, 2):
                    rsl = slice(j * P, (j + 2) * P)
                    nc.sync.dma_start(
                        vnat[:, j:j + 2, :],
                        ins["vn"][b, rsl, :].rearrange("(nk p) h -> p nk h", p=P))
                vnat_tiles.clear()
                vnat_tiles[b] = vnat
            vnat = vnat_tiles[b]
            return hT, vnat

        def emit_stage_b(b, lt, apart, last=False):
            hT, vnat = apart
            # Stage B: expT[o, l] = exp(Vw h + vb)  (no max-subtraction; scores
            # are small).
            # final tile: expT is f32r so the softmax sum can accumulate on
            # the PE right behind each exp, keeping the tail's DVE chain short
            expT = epool.tile([P, NH, LT], F32R if last else F32, tag="expT")
            ps = psS.tile([P, LT], F32, tag="ps")
            for o in range(NH):
                osl = slice(o * P, (o + 1) * P)
                pb = psB.tile([P, LT], F32, tag="pb")
                for ht in range(NH):
                    nc.tensor.matmul(pb, vws[:, ht, osl], hT[:, ht, :],
                                     start=(ht == 0), stop=(ht == NH - 1))
                nc.scalar.activation(expT[:, o, :], pb, AFT.Exp,
                                     bias=vbs[:, o:o + 1], scale=1.0)
                if last:
                    nc.tensor.matmul(ps, ones[:], expT[:, o, :],
                                     start=(o == 0), stop=(o == NH - 1))
            if not last:
                # Partition-dim softmax sums, replicated to all partitions:
                # DVE tree-sum over the 8 o-blocks, then one ones-matmul to
                # replicate the 128 partial sums across partitions.
                t4 = mpool.tile([P, 4, LT], F32, tag="t4")
                t2 = mpool.tile([P, 2, LT], F32, tag="t2")
                s1 = mpool.tile([P, LT], F32R, tag="s1")
                nc.vector.tensor_add(t4, expT[:, 0:4, :], expT[:, 4:8, :])
                nc.vector.tensor_add(t2, t4[:, 0:2, :], t4[:, 2:4, :])
                nc.vector.tensor_add(s1, t2[:, 0, :], t2[:, 1, :])
                nc.tensor.matmul(ps, ones[:], s1[:], start=True, stop=True)
            return (b, lt, expT, ps, vnat)

        def emit_softmax_context(state, last=False):
            b, lt, expT, ps, vnat = state
            lsl = slice(lt * LT, (lt + 1) * LT)
            # last tile's expT is f32r-typed; read through an f32 view, write
            # in-place through the f32r-typed AP (BIR writers-rounded rule)
            expf = expT[:].bitcast(F32) if last else expT
            recip = mpool.tile([P, LT], F32, tag="recip")
            rscr = mpool.tile([P, LT], F32, tag="rscr")
            # ~2 ULP, ~2.8x faster than reciprocal(); sums are ~1e3 so no edge cases
            nc.vector.reciprocal_approx_accurate(recip, ps, rscr)
            attw = apool.tile([P, NH, LT], BF16, tag="attw")
            # all bf16 attw muls first: they gate the context matmuls on PE.
            # On the last step (nothing left to hide the DVE chain behind) do
            # them in l-halves so the first context groups start sooner.
            halves = [slice(0, LT // 2), slice(LT // 2, LT)] if last \
                else [slice(0, LT)]
            for hsl2 in halves:
                for o in range(NH):
                    nc.vector.tensor_mul(attw[:, o, hsl2], expf[:, o, hsl2],
                                         recip[:, hsl2])
            for o in range(NH):
                nc.vector.tensor_mul(expT[:, o, :], expf[:, o, :], recip)
            nc.sync.dma_start(
                attT[b, :, lsl].rearrange("(nh p) l -> p nh l", p=P), expf)

            # Context: ctx[l, h] = sum_k att[k, l] * v[k, h]
            for lb in range(LT // P):
                row0 = lt * LT + lb * P
                for hti in range(NHT):
                    hsl = slice(hti * LT, (hti + 1) * LT)
                    pc = psC.tile([P, LT], F32, tag="pc")
                    for kt in range(NH):
                        nc.tensor.matmul(pc, attw[:, kt, lb * P:(lb + 1) * P],
                                         vnat[:, kt, hsl],
                                         start=(kt == 0), stop=(kt == NH - 1))
                    cs = cpool.tile([P, LT], BF16, tag="cs")
                    # PSUM->SBUF evacuation alternating ScalarE/DVE so neither
                    # queue's backlog blocks psC slot reuse for long
                    if hti == 0:
                        nc.scalar.activation(cs, pc, AFT.Copy)
                    else:
                        nc.vector.tensor_copy(cs, pc)
                    nc.sync.dma_start(ctxo[b, row0:row0 + P, hsl], cs)

        pending = None
        nsteps = len(steps)
        for i, (b, lt) in enumerate(steps):
            apart = emit_stage_a(
                b, lt, preloaded=(qv0, qb0, vb0) if i == 0 else None)
            state = emit_stage_b(b, lt, apart, last=(i == nsteps - 1))
            if pending is not None:
                emit_softmax_context(pending)
            pending = state
        emit_softmax_context(pending, last=True)


def _get_program():
    if "nc" not in _PROGRAM_CACHE:
        _PROGRAM_CACHE["nc"] = _build_program()
    return _PROGRAM_CACHE["nc"]


def _prep_in_maps(query, value, w1_w, w1_b, w2_w, w2_b, v_w, v_b):
    bf16 = ml_dtypes.bfloat16
    fp8 = ml_dtypes.float8_e4m3
    # [h, o] layouts; fp8 rows scaled by WS, bf16 rows too (exact power of
    # two) so the PSUM accumulator has one uniform scale.
    w1t = np.ascontiguousarray(w1_w.T) * WS
    w2t = np.ascontiguousarray(w2_w.T) * WS
    vwt = v_w.T.astype(bf16)
    b12 = np.ascontiguousarray((w1_b + w2_b).astype(np.float32).reshape(NH, P).T)
    vbt = np.ascontiguousarray(v_b.astype(np.float32).reshape(NH, P).T)

    base = {
        "vwt_in": vwt,
        "b12_in": b12,
        "vbt_in": vbt,
        "ones_in": np.ones((P, P), np.float32),
    }
    if NF8:
        base["w18_in"] = w1t[:NF8 * P].astype(fp8)
        base["w28_in"] = w2t[:NF8 * P].astype(fp8)
    if NR:
        base["w1b_in"] = w1t[NF8 * P:].astype(bf16)
        base["w2b_in"] = w2t[NF8 * P:].astype(bf16)

    in_maps = []
    for c in range(NCORES):
        sl = slice(c * BLOC, (c + 1) * BLOC)
        qT = query[sl].transpose(0, 2, 1) * QS
        vT = value[sl].transpose(0, 2, 1) * QS
        m = dict(base)
        if NF8:
            m["q8_in"] = qT[:, :NF8 * P].astype(fp8)
            m["v8_in"] = vT[:, :NF8 * P].astype(fp8)
        if NR:
            m["qb_in"] = qT[:, NF8 * P:].astype(bf16)
            m["vb16_in"] = vT[:, NF8 * P:].astype(bf16)
        m["vn_in"] = value[sl].astype(bf16)
        in_maps.append(m)
    return in_maps


def run_sharded(inputs, **run_kwargs):
    """Build in_maps, run on 8 cores, return (att, ctx, BassKernelResults)."""
    query = np.asarray(inputs["query"], dtype=np.float32)
    value = np.asarray(inputs["value"], dtype=np.float32)
    in_maps = _prep_in_maps(
        query, value,
        np.asarray(inputs["w1_w"], np.float32), np.asarray(inputs["w1_b"], np.float32),
        np.asarray(inputs["w2_w"], np.float32), np.asarray(inputs["w2_b"], np.float32),
        np.asarray(inputs["v_w"], np.float32), np.asarray(inputs["v_b"], np.float32),
    )
    nc = _get_program()
    res = bass_utils.run_bass_kernel_spmd(
        nc, in_maps, core_ids=list(range(NCORES)), **run_kwargs)

    att = np.empty((B, L, H), np.float32)
    ctxv = np.empty((B, L, H), np.float32)
    for c in range(NCORES):
        sl = slice(c * BLOC, (c + 1) * BLOC)
        att[sl] = res.results[c]["att_out"].transpose(0, 2, 1)
        ctxv[sl] = res.results[c]["ctx_out"].astype(np.float32)
    return att, ctxv, res


def kernel(**inputs):
    att, ctxv, _ = run_sharded(inputs)
    return att, ctxv


# revision 16
# speedup vs baseline: 1.0091x; 1.0019x over previous
"""Bahdanau attention kernel for Trainium2, 8-core data-parallel.

Problem (B=32, L=1024, H=1024, fp32):
    h     = tanh(q @ W1.T + b1 + v @ W2.T + b2)        # (B, L, H)
    score = h @ Vw.T + vb                              # (B, L, H)
    att   = softmax(score, axis=-1)                    # (B, L, H)
    ctx   = att @ v                                    # (B, L, H)  (bmm over kv dim)
    returns (att, ctx)

Strategy:
  - Data-parallel: 4 batches per core on 8 cores.
  - Everything on-device runs in a TRANSPOSED layout [h, l] so that the
    contraction dim (h / k) always lands on SBUF partitions and no on-device
    transposes are needed. The host pre-transposes q and value per batch
    (and pre-transposes the weight matrices), and transposes the attention
    weights output back after gathering.
  - A tunable slice of stage A (the q@W1 + v@W2 preactivation) runs in fp8e4
    with perf_mode=DoubleRow: one matmul contracts the j-th 128-block of BOTH
    streams (W1_j x q_j paired with W2_j x v_j) at bf16 column rate, i.e. 2x
    PE throughput for that slice. The fp8 fraction is the accuracy/speed
    knob: softmax absmax error grows as sqrt(fraction) of ~3.0e-2 (measured
    at fraction 1), and the harness gate is 2e-2. Host pre-scales q/v by 2^4
    and W1/W2 by 2^6 to keep fp8 in the normal range (bf16-path operands get
    the same exact power-of-two scales so the PSUM accumulator is uniform);
    the tanh activation rescales by 2^-10. Stage B and the context matmul
    stay bf16.
  - The partition-dim softmax sum is a DVE add-tree (8 o-blocks -> 1) plus a
    single replicating ones-matmul instead of 8 accumulated ones-matmuls.
  - Per (batch, l-tile of 512): stage A -> tanh -> stage B (8 x 8 accums) ->
    exp -> DVE sum tree -> ones-matmul -> reciprocal -> normalize -> context
    matmul (8 x 8 accums). Softmax+context of step i is emitted after the
    matmul stages of step i+1 so the PE never waits on DVE work.
"""

import numpy as np
import ml_dtypes
from contextlib import ExitStack

import concourse.bass as bass
import concourse.mybir as mybir
import concourse.tile as tile
from concourse import bacc, bass_utils

B, L, H = 32, 1024, 1024
NCORES = 8
BLOC = B // NCORES  # batches per core
P = 128             # partitions
LT = 512            # l-tile (moving free dim)
NLT = L // LT       # l-tiles per batch
NH = H // P         # 128-blocks along h / o / k
NHT = H // LT       # 512-tiles along h (context output)

# Stage-A precision knob: number of 128-blocks (of 8) of the contraction
# whose q AND v streams run in one fp8 DoubleRow matmul; the remaining
# blocks run as two bf16 matmuls each.
NF8 = 3
NR = NH - NF8                 # bf16 remainder blocks
QS = 16.0                     # pre-scale on q/v
WS = 64.0                     # pre-scale on w1/w2
ASCALE = 1.0 / (QS * WS)      # tanh activation rescale

BF16 = mybir.dt.bfloat16
F32 = mybir.dt.float32
F32R = mybir.dt.float32r
FP8 = mybir.dt.float8e4
AFT = mybir.ActivationFunctionType
DR = mybir.MatmulPerfMode.DoubleRow

_PROGRAM_CACHE = {}


def _build_program():
    nc = bacc.Bacc("TRN2", target_bir_lowering=False, debug=False)

    def din(name, shape, dt):
        return nc.dram_tensor(name, shape, dt, kind="ExternalInput").ap()

    ins = {}
    if NF8:
        ins["q8"] = din("q8_in", [BLOC, NF8 * P, L], FP8)
        ins["v8"] = din("v8_in", [BLOC, NF8 * P, L], FP8)
        ins["w18"] = din("w18_in", [NF8 * P, H], FP8)
        ins["w28"] = din("w28_in", [NF8 * P, H], FP8)
    if NF8 < NH:
        ins["qb"] = din("qb_in", [BLOC, NR * P, L], BF16)
        ins["vb16"] = din("vb16_in", [BLOC, NR * P, L], BF16)
        ins["w1b"] = din("w1b_in", [NR * P, H], BF16)
        ins["w2b"] = din("w2b_in", [NR * P, H], BF16)
    ins["vn"] = din("vn_in", [BLOC, L, H], BF16)
    ins["vwt"] = din("vwt_in", [H, H], BF16)
    ins["b12"] = din("b12_in", [P, NH], F32)
    ins["vbt"] = din("vbt_in", [P, NH], F32)
    # float32r ones for the partition-dim softmax sum (memset can't write f32r)
    ins["onesd"] = din("ones_in", [P, P], F32R)

    attT = nc.dram_tensor("att_out", [BLOC, H, L], F32, kind="ExternalOutput").ap()
    # bf16 context output (host upcasts): halves the ctx SBUF-read + HBM-write
    # DMA traffic; ctx error budget has ~2x headroom vs the att one
    ctxo = nc.dram_tensor("ctx_out", [BLOC, L, H], BF16, kind="ExternalOutput").ap()

    with tile.TileContext(nc) as tc:
        _kernel_body(tc, ins, attT, ctxo)
    nc.compile()
    return nc


def _kernel_body(tc, ins, attT, ctxo):
    nc = tc.nc
    with ExitStack() as ctx:
        consts = ctx.enter_context(tc.tile_pool(name="consts", bufs=1))
        qpool = ctx.enter_context(tc.tile_pool(name="qpool", bufs=2))
        hpool = ctx.enter_context(tc.tile_pool(name="hpool", bufs=2))
        epool = ctx.enter_context(tc.tile_pool(name="epool", bufs=2))
        apool = ctx.enter_context(tc.tile_pool(name="apool", bufs=2))
        vpool = ctx.enter_context(tc.tile_pool(name="vpool", bufs=2))
        mpool = ctx.enter_context(tc.tile_pool(name="mpool", bufs=2))
        cpool = ctx.enter_context(tc.tile_pool(name="cpool", bufs=3))
        psA = ctx.enter_context(tc.tile_pool(name="psA", bufs=2, space="PSUM"))
        psB = ctx.enter_context(tc.tile_pool(name="psB", bufs=2, space="PSUM"))
        psS = ctx.enter_context(tc.tile_pool(name="psS", bufs=1, space="PSUM"))
        psC = ctx.enter_context(tc.tile_pool(name="psC", bufs=3, space="PSUM"))

        def load_qv(b, lt):
            """Allocate + chunk-DMA the transposed q/v slabs for one l-tile.
            qv8 interleaves (q_j, v_j) per fp8 block for DoubleRow pairing."""
            lsl = slice(lt * LT, (lt + 1) * LT)
            qv8 = qpool.tile([P, NF8, 2, LT], FP8, tag="qv8", name="qv8") \
                if NF8 else None
            qsb = qpool.tile([P, NR, LT], BF16, tag="qsb", name="qsb") \
                if NR else None
            vsb = qpool.tile([P, NR, LT], BF16, tag="vsb", name="vsb") \
                if NR else None
            for j in range(NF8):
                rsl = slice(j * P, (j + 1) * P)
                nc.sync.dma_start(qv8[:, j, 0, :], ins["q8"][b, rsl, lsl])
                nc.sync.dma_start(qv8[:, j, 1, :], ins["v8"][b, rsl, lsl])
            for j in range(0, NR, 2):
                j2 = min(j + 2, NR)
                rsl = slice(j * P, j2 * P)
                nc.sync.dma_start(
                    qsb[:, j:j2, :],
                    ins["qb"][b, rsl, lsl].rearrange("(nh p) l -> p nh l", p=P))
                nc.sync.dma_start(
                    vsb[:, j:j2, :],
                    ins["vb16"][b, rsl, lsl].rearrange("(nh p) l -> p nh l", p=P))
            return qv8, qsb, vsb

        # Resident stage-A weights, contraction 128-block on partitions; fp8
        # (w1_j, w2_j) interleaved pairs first, bf16 remainder after. Chunk
        # loads are interleaved with step 0's q/v chunks in consumption order
        # so the first matmul gates on ~0.5 MB.
        # HAM pre-warm: ~10 dummy matmuls on zeroed scratch run during the
        # startup DMA wait so the PE clock gate is already 8/8 (2.4 GHz) when
        # the first real matmul issues (saves the ~3.4us half-clock ramp).
        warm = consts.tile([P, LT], BF16, name="warm")
        nc.any.memset(warm, 0)
        pwarm = psC.tile([P, LT], F32, tag="pc", name="pwarm")
        for k in range(10):
            nc.tensor.matmul(pwarm, warm[:, 0:P], warm[:],
                             start=(k == 0), stop=(k == 9))
        b12s = consts.tile([P, NH], F32)
        nc.sync.dma_start(b12s, ins["b12"])
        vbs = consts.tile([P, NH], F32)
        nc.sync.dma_start(vbs, ins["vbt"])
        ones = consts.tile([P, P], F32R)
        nc.sync.dma_start(ones, ins["onesd"])
        w128 = consts.tile([P, NF8, 2, H], FP8, name="w128") if NF8 else None
        w1sb = consts.tile([P, NR, H], BF16, name="w1sb") if NR else None
        w2sb = consts.tile([P, NR, H], BF16, name="w2sb") if NR else None
        qv0 = qpool.tile([P, NF8, 2, LT], FP8, tag="qv8", name="qv8") \
            if NF8 else None
        qb0 = qpool.tile([P, NR, LT], BF16, tag="qsb", name="qsb") if NR else None
        vb0 = qpool.tile([P, NR, LT], BF16, tag="vsb", name="vsb") if NR else None
        for j in range(NF8):
            rsl = slice(j * P, (j + 1) * P)
            nc.sync.dma_start(w128[:, j, 0, :], ins["w18"][rsl, :])
            nc.sync.dma_start(w128[:, j, 1, :], ins["w28"][rsl, :])
            nc.sync.dma_start(qv0[:, j, 0, :], ins["q8"][0, rsl, 0:LT])
            nc.sync.dma_start(qv0[:, j, 1, :], ins["v8"][0, rsl, 0:LT])
        for j in range(0, NR, 2):
            j2 = min(j + 2, NR)
            rsl = slice(j * P, j2 * P)
            nc.scalar.dma_start(w1sb[:, j:j2, :],
                                ins["w1b"][rsl, :].rearrange("(nh p) o -> p nh o", p=P))
            nc.scalar.dma_start(w2sb[:, j:j2, :],
                                ins["w2b"][rsl, :].rearrange("(nh p) o -> p nh o", p=P))
            nc.scalar.dma_start(
                qb0[:, j:j2, :],
                ins["qb"][0, rsl, 0:LT].rearrange("(nh p) l -> p nh l", p=P))
            nc.scalar.dma_start(
                vb0[:, j:j2, :],
                ins["vb16"][0, rsl, 0:LT].rearrange("(nh p) l -> p nh l", p=P))
        vws = consts.tile([P, NH, H], BF16)
        vws_loaded = []

        steps = [(b, lt) for b in range(BLOC) for lt in range(NLT)]
        vnat_tiles = {}

        def emit_stage_a(b, lt, preloaded=None):
            qv8, qsb, vsb = preloaded if preloaded is not None else load_qv(b, lt)

            # Stage A: hT[o, l] = tanh((W1' q'^T + W2' v'^T) * 2^-10 + b1 + b2)
            # fp8 DoubleRow blocks contract q_j AND v_j in one matmul.
            hT = hpool.tile([P, NH, LT], BF16, tag="hT")
            for o in range(NH):
                osl = slice(o * P, (o + 1) * P)
                pa = psA.tile([P, LT], F32, tag="pa")
                for j in range(NF8):
                    nc.tensor.matmul(pa, w128[:, j, :, osl], qv8[:, j, :, :],
                                     start=(j == 0), stop=(NR == 0 and j == NF8 - 1),
                                     perf_mode=DR)
                for j in range(NR):
                    nc.tensor.matmul(pa, w1sb[:, j, osl], qsb[:, j, :],
                                     start=(NF8 == 0 and j == 0), stop=False)
                    nc.tensor.matmul(pa, w2sb[:, j, osl], vsb[:, j, :],
                                     start=False, stop=(j == NR - 1))
                nc.scalar.activation(hT[:, o, :], pa, AFT.Tanh,
                                     bias=b12s[:, o:o + 1], scale=ASCALE)

            if not vws_loaded:
                for ht in range(NH):
                    nc.sync.dma_start(vws[:, ht, :],
                                      ins["vwt"][ht * P:(ht + 1) * P, :])
                vws_loaded.append(True)

            # value in natural [k, h] layout for the context matmul (used ~a full
            # step later, so the DMA is emitted after stage A's)
            if b not in vnat_tiles:
                vnat = vpool.tile([P, NH, H], BF16, tag="vnat")
                for j in range(0, NH, 2):
                    rsl = slice(j * P, (j + 2) * P)
                    nc.sync.dma_start(
                        vnat[:, j:j + 2, :],
                        ins["vn"][b, rsl, :].rearrange("(nk p) h -> p nk h", p=P))
                vnat_tiles.clear()
                vnat_tiles[b] = vnat
            vnat = vnat_tiles[b]
            return hT, vnat

        def emit_stage_b(b, lt, apart, last=False):
            hT, vnat = apart
            # Stage B: expT[o, l] = exp(Vw h + vb)  (no max-subtraction; scores
            # are small).
            # final tile: expT is f32r so the softmax sum can accumulate on
            # the PE right behind each exp, keeping the tail's DVE chain short
            expT = epool.tile([P, NH, LT], F32R if last else F32, tag="expT")
            ps = psS.tile([P, LT], F32, tag="ps")
            for o in range(NH):
                osl = slice(o * P, (o + 1) * P)
                pb = psB.tile([P, LT], F32, tag="pb")
                for ht in range(NH):
                    nc.tensor.matmul(pb, vws[:, ht, osl], hT[:, ht, :],
                                     start=(ht == 0), stop=(ht == NH - 1))
                nc.scalar.activation(expT[:, o, :], pb, AFT.Exp,
                                     bias=vbs[:, o:o + 1], scale=1.0)
                if last:
                    nc.tensor.matmul(ps, ones[:], expT[:, o, :],
                                     start=(o == 0), stop=(o == NH - 1))
            if not last:
                # Partition-dim softmax sums, replicated to all partitions:
                # DVE tree-sum over the 8 o-blocks, then one ones-matmul to
                # replicate the 128 partial sums across partitions.
                t4 = mpool.tile([P, 4, LT], F32, tag="t4")
                t2 = mpool.tile([P, 2, LT], F32, tag="t2")
                s1 = mpool.tile([P, LT], F32R, tag="s1")
                nc.vector.tensor_add(t4, expT[:, 0:4, :], expT[:, 4:8, :])
                nc.vector.tensor_add(t2, t4[:, 0:2, :], t4[:, 2:4, :])
                nc.vector.tensor_add(s1, t2[:, 0, :], t2[:, 1, :])
                nc.tensor.matmul(ps, ones[:], s1[:], start=True, stop=True)
            return (b, lt, expT, ps, vnat)

        def emit_softmax_context(state, last=False):
            b, lt, expT, ps, vnat = state
            lsl = slice(lt * LT, (lt + 1) * LT)
            # last tile's expT is f32r-typed; read through an f32 view, write
            # in-place through the f32r-typed AP (BIR writers-rounded rule)
            expf = expT[:].bitcast(F32) if last else expT
            recip = mpool.tile([P, LT], F32, tag="recip")
            rscr = mpool.tile([P, LT], F32, tag="rscr")
            # ~2 ULP, ~2.8x faster than reciprocal(); sums are ~1e3 so no edge cases
            nc.vector.reciprocal_approx_accurate(recip, ps, rscr)
            attw = apool.tile([P, NH, LT], BF16, tag="attw")
            # all bf16 attw muls first: they gate the context matmuls on PE.
            # On the last step (nothing left to hide the DVE chain behind) do
            # them in l-halves so the first context groups start sooner.
            halves = [slice(0, LT // 2), slice(LT // 2, LT)] if last \
                else [slice(0, LT)]
            for hsl2 in halves:
                for o in range(NH):
                    nc.vector.tensor_mul(attw[:, o, hsl2], expf[:, o, hsl2],
                                         recip[:, hsl2])
            for o in range(NH):
                nc.vector.tensor_mul(expT[:, o, :], expf[:, o, :], recip)
            nc.sync.dma_start(
                attT[b, :, lsl].rearrange("(nh p) l -> p nh l", p=P), expf)

            # Context: ctx[l, h] = sum_k att[k, l] * v[k, h]
            for lb in range(LT // P):
                row0 = lt * LT + lb * P
                for hti in range(NHT):
                    hsl = slice(hti * LT, (hti + 1) * LT)
                    pc = psC.tile([P, LT], F32, tag="pc")
                    for kt in range(NH):
                        nc.tensor.matmul(pc, attw[:, kt, lb * P:(lb + 1) * P],
                                         vnat[:, kt, hsl],
                                         start=(kt == 0), stop=(kt == NH - 1))
                    cs = cpool.tile([P, LT], BF16, tag="cs")
                    # PSUM->SBUF evacuation alternating ScalarE/DVE so neither
                    # queue's backlog blocks psC slot reuse for long
                    if hti == 0:
                        nc.scalar.activation(cs, pc, AFT.Copy)
                    else:
                        nc.vector.tensor_copy(cs, pc)
                    nc.sync.dma_start(ctxo[b, row0:row0 + P, hsl], cs)

        pending = None
        nsteps = len(steps)
        for i, (b, lt) in enumerate(steps):
            apart = emit_stage_a(
                b, lt, preloaded=(qv0, qb0, vb0) if i == 0 else None)
            state = emit_stage_b(b, lt, apart, last=(i == nsteps - 1))
            if pending is not None:
                emit_softmax_context(pending)
            pending = state
        emit_softmax_context(pending, last=True)


def _get_program():
    if "nc" not in _PROGRAM_CACHE:
        _PROGRAM_CACHE["nc"] = _build_program()
    return _PROGRAM_CACHE["nc"]


def _prep_in_maps(query, value, w1_w, w1_b, w2_w, w2_b, v_w, v_b):
    bf16 = ml_dtypes.bfloat16
    fp8 = ml_dtypes.float8_e4m3
    # [h, o] layouts; fp8 rows scaled by WS, bf16 rows too (exact power of
    # two) so the PSUM accumulator has one uniform scale.
    w1t = np.ascontiguousarray(w1_w.T) * WS
    w2t = np.ascontiguousarray(w2_w.T) * WS
    vwt = v_w.T.astype(bf16)
    b12 = np.ascontiguousarray((w1_b + w2_b).astype(np.float32).reshape(NH, P).T)
    vbt = np.ascontiguousarray(v_b.astype(np.float32).reshape(NH, P).T)

    base = {
        "vwt_in": vwt,
        "b12_in": b12,
        "vbt_in": vbt,
        "ones_in": np.ones((P, P), np.float32),
    }
    if NF8:
        base["w18_in"] = w1t[:NF8 * P].astype(fp8)
        base["w28_in"] = w2t[:NF8 * P].astype(fp8)
    if NR:
        base["w1b_in"] = w1t[NF8 * P:].astype(bf16)
        base["w2b_in"] = w2t[NF8 * P:].astype(bf16)

    in_maps = []
    for c in range(NCORES):
        sl = slice(c * BLOC, (c + 1) * BLOC)
        qT = query[sl].transpose(0, 2, 1) * QS
        vT = value[sl].transpose(0, 2, 1) * QS
        m = dict(base)
        if NF8:
            m["q8_in"] = qT[:, :NF8 * P].astype(fp8)
            m["v8_in"] = vT[:, :NF8 * P].astype(fp8)
        if NR:
            m["qb_in"] = qT[:, NF8 * P:].astype(bf16)
            m["vb16_in"] = vT[:, NF8 * P:].astype(bf16)
        m["vn_in"] = value[sl].astype(bf16)
        in_maps.append(m)
    return in_maps


def run_sharded(inputs, **run_kwargs):
    """Build in_maps, run on 8 cores, return (att, ctx, BassKernelResults)."""
    query = np.asarray(inputs["query"], dtype=np.float32)
    value = np.asarray(inputs["value"], dtype=np.float32)
    in_maps = _prep_in_maps(
        query, value,
        np.asarray(inputs["w1_w"], np.float32), np.asarray(inputs["w1_b"], np.float32),
        np.asarray(inputs["w2_w"], np.float32), np.asarray(inputs["w2_b"], np.float32),
        np.asarray(inputs["v_w"], np.float32), np.asarray(inputs["v_b"], np.float32),
    )
    nc = _get_program()
    res = bass_utils.run_bass_kernel_spmd(
        nc, in_maps, core_ids=list(range(NCORES)), **run_kwargs)

    att = np.empty((B, L, H), np.float32)
    ctxv = np.empty((B, L, H), np.float32)
    for c in range(NCORES):
        sl = slice(c * BLOC, (c + 1) * BLOC)
        att[sl] = res.results[c]["att_out"].transpose(0, 2, 1)
        ctxv[sl] = res.results[c]["ctx_out"].astype(np.float32)
    return att, ctxv, res


def kernel(**inputs):
    att, ctxv, _ = run_sharded(inputs)
    return att, ctxv


# revision 17
# speedup vs baseline: 1.0108x; 1.0017x over previous
"""Bahdanau attention kernel for Trainium2, 8-core data-parallel.

Problem (B=32, L=1024, H=1024, fp32):
    h     = tanh(q @ W1.T + b1 + v @ W2.T + b2)        # (B, L, H)
    score = h @ Vw.T + vb                              # (B, L, H)
    att   = softmax(score, axis=-1)                    # (B, L, H)
    ctx   = att @ v                                    # (B, L, H)  (bmm over kv dim)
    returns (att, ctx)

Strategy:
  - Data-parallel: 4 batches per core on 8 cores.
  - Everything on-device runs in a TRANSPOSED layout [h, l] so that the
    contraction dim (h / k) always lands on SBUF partitions and no on-device
    transposes are needed. The host pre-transposes q and value per batch
    (and pre-transposes the weight matrices), and transposes the attention
    weights output back after gathering.
  - A tunable slice of stage A (the q@W1 + v@W2 preactivation) runs in fp8e4
    with perf_mode=DoubleRow: one matmul contracts the j-th 128-block of BOTH
    streams (W1_j x q_j paired with W2_j x v_j) at bf16 column rate, i.e. 2x
    PE throughput for that slice. The fp8 fraction is the accuracy/speed
    knob: softmax absmax error grows as sqrt(fraction) of ~3.0e-2 (measured
    at fraction 1), and the harness gate is 2e-2. Host pre-scales q/v by 2^4
    and W1/W2 by 2^6 to keep fp8 in the normal range (bf16-path operands get
    the same exact power-of-two scales so the PSUM accumulator is uniform);
    the tanh activation rescales by 2^-10. Stage B and the context matmul
    stay bf16.
  - The partition-dim softmax sum is a DVE add-tree (8 o-blocks -> 1) plus a
    single replicating ones-matmul instead of 8 accumulated ones-matmuls.
  - Per (batch, l-tile of 512): stage A -> tanh -> stage B (8 x 8 accums) ->
    exp -> DVE sum tree -> ones-matmul -> reciprocal -> normalize -> context
    matmul (8 x 8 accums). Softmax+context of step i is emitted after the
    matmul stages of step i+1 so the PE never waits on DVE work.
"""

import numpy as np
import ml_dtypes
from contextlib import ExitStack

import concourse.bass as bass
import concourse.mybir as mybir
import concourse.tile as tile
from concourse import bacc, bass_utils

B, L, H = 32, 1024, 1024
NCORES = 8
BLOC = B // NCORES  # batches per core
P = 128             # partitions
LT = 512            # l-tile (moving free dim)
NLT = L // LT       # l-tiles per batch
NH = H // P         # 128-blocks along h / o / k
NHT = H // LT       # 512-tiles along h (context output)

# Stage-A precision knob: number of 128-blocks (of 8) of the contraction
# whose q AND v streams run in one fp8 DoubleRow matmul; the remaining
# blocks run as two bf16 matmuls each.
NF8 = 3
NR = NH - NF8                 # bf16 remainder blocks
QS = 16.0                     # pre-scale on q/v
WS = 64.0                     # pre-scale on w1/w2
ASCALE = 1.0 / (QS * WS)      # tanh activation rescale

BF16 = mybir.dt.bfloat16
F32 = mybir.dt.float32
F32R = mybir.dt.float32r
FP8 = mybir.dt.float8e4
AFT = mybir.ActivationFunctionType
DR = mybir.MatmulPerfMode.DoubleRow

_PROGRAM_CACHE = {}


def _build_program():
    nc = bacc.Bacc("TRN2", target_bir_lowering=False, debug=False)

    def din(name, shape, dt):
        return nc.dram_tensor(name, shape, dt, kind="ExternalInput").ap()

    ins = {}
    if NF8:
        ins["q8"] = din("q8_in", [BLOC, NF8 * P, L], FP8)
        ins["v8"] = din("v8_in", [BLOC, NF8 * P, L], FP8)
        ins["w18"] = din("w18_in", [NF8 * P, H], FP8)
        ins["w28"] = din("w28_in", [NF8 * P, H], FP8)
    if NF8 < NH:
        ins["qb"] = din("qb_in", [BLOC, NR * P, L], BF16)
        ins["vb16"] = din("vb16_in", [BLOC, NR * P, L], BF16)
        ins["w1b"] = din("w1b_in", [NR * P, H], BF16)
        ins["w2b"] = din("w2b_in", [NR * P, H], BF16)
    ins["vn"] = din("vn_in", [BLOC, L, H], BF16)
    ins["vwt"] = din("vwt_in", [H, H], BF16)
    ins["b12"] = din("b12_in", [P, NH], F32)
    ins["vbt"] = din("vbt_in", [P, NH], F32)
    # float32r ones for the partition-dim softmax sum (memset can't write f32r)
    ins["onesd"] = din("ones_in", [P, P], F32R)

    attT = nc.dram_tensor("att_out", [BLOC, H, L], F32, kind="ExternalOutput").ap()
    # bf16 context output (host upcasts): halves the ctx SBUF-read + HBM-write
    # DMA traffic; ctx error budget has ~2x headroom vs the att one
    ctxo = nc.dram_tensor("ctx_out", [BLOC, L, H], BF16, kind="ExternalOutput").ap()

    with tile.TileContext(nc) as tc:
        _kernel_body(tc, ins, attT, ctxo)
    nc.compile()
    return nc


def _kernel_body(tc, ins, attT, ctxo):
    nc = tc.nc
    with ExitStack() as ctx:
        consts = ctx.enter_context(tc.tile_pool(name="consts", bufs=1))
        qpool = ctx.enter_context(tc.tile_pool(name="qpool", bufs=2))
        hpool = ctx.enter_context(tc.tile_pool(name="hpool", bufs=2))
        epool = ctx.enter_context(tc.tile_pool(name="epool", bufs=2))
        apool = ctx.enter_context(tc.tile_pool(name="apool", bufs=2))
        vpool = ctx.enter_context(tc.tile_pool(name="vpool", bufs=2))
        mpool = ctx.enter_context(tc.tile_pool(name="mpool", bufs=2))
        cpool = ctx.enter_context(tc.tile_pool(name="cpool", bufs=3))
        psA = ctx.enter_context(tc.tile_pool(name="psA", bufs=2, space="PSUM"))
        psB = ctx.enter_context(tc.tile_pool(name="psB", bufs=2, space="PSUM"))
        psS = ctx.enter_context(tc.tile_pool(name="psS", bufs=1, space="PSUM"))
        psC = ctx.enter_context(tc.tile_pool(name="psC", bufs=3, space="PSUM"))

        def load_qv(b, lt):
            """Allocate + chunk-DMA the transposed q/v slabs for one l-tile.
            qv8 interleaves (q_j, v_j) per fp8 block for DoubleRow pairing."""
            lsl = slice(lt * LT, (lt + 1) * LT)
            qv8 = qpool.tile([P, NF8, 2, LT], FP8, tag="qv8", name="qv8") \
                if NF8 else None
            qsb = qpool.tile([P, NR, LT], BF16, tag="qsb", name="qsb") \
                if NR else None
            vsb = qpool.tile([P, NR, LT], BF16, tag="vsb", name="vsb") \
                if NR else None
            for j in range(NF8):
                rsl = slice(j * P, (j + 1) * P)
                nc.sync.dma_start(qv8[:, j, 0, :], ins["q8"][b, rsl, lsl])
                nc.sync.dma_start(qv8[:, j, 1, :], ins["v8"][b, rsl, lsl])
            for j in range(0, NR, 2):
                j2 = min(j + 2, NR)
                rsl = slice(j * P, j2 * P)
                nc.sync.dma_start(
                    qsb[:, j:j2, :],
                    ins["qb"][b, rsl, lsl].rearrange("(nh p) l -> p nh l", p=P))
                nc.sync.dma_start(
                    vsb[:, j:j2, :],
                    ins["vb16"][b, rsl, lsl].rearrange("(nh p) l -> p nh l", p=P))
            return qv8, qsb, vsb

        # Resident stage-A weights, contraction 128-block on partitions; fp8
        # (w1_j, w2_j) interleaved pairs first, bf16 remainder after. Chunk
        # loads are interleaved with step 0's q/v chunks in consumption order
        # so the first matmul gates on ~0.5 MB.
        # HAM pre-warm: ~10 dummy matmuls on zeroed scratch run during the
        # startup DMA wait so the PE clock gate is already 8/8 (2.4 GHz) when
        # the first real matmul issues (saves the ~3.4us half-clock ramp).
        warm = consts.tile([P, LT], BF16, name="warm")
        nc.any.memset(warm, 0)
        pwarm = psC.tile([P, LT], F32, tag="pc", name="pwarm")
        for k in range(10):
            nc.tensor.matmul(pwarm, warm[:, 0:P], warm[:],
                             start=(k == 0), stop=(k == 9))
        b12s = consts.tile([P, NH], F32)
        nc.sync.dma_start(b12s, ins["b12"])
        vbs = consts.tile([P, NH], F32)
        nc.sync.dma_start(vbs, ins["vbt"])
        ones = consts.tile([P, P], F32R)
        nc.sync.dma_start(ones, ins["onesd"])
        w128 = consts.tile([P, NF8, 2, H], FP8, name="w128") if NF8 else None
        w1sb = consts.tile([P, NR, H], BF16, name="w1sb") if NR else None
        w2sb = consts.tile([P, NR, H], BF16, name="w2sb") if NR else None
        qv0 = qpool.tile([P, NF8, 2, LT], FP8, tag="qv8", name="qv8") \
            if NF8 else None
        qb0 = qpool.tile([P, NR, LT], BF16, tag="qsb", name="qsb") if NR else None
        vb0 = qpool.tile([P, NR, LT], BF16, tag="vsb", name="vsb") if NR else None
        for j in range(NF8):
            rsl = slice(j * P, (j + 1) * P)
            nc.sync.dma_start(w128[:, j, 0, :], ins["w18"][rsl, :])
            nc.sync.dma_start(w128[:, j, 1, :], ins["w28"][rsl, :])
            nc.sync.dma_start(qv0[:, j, 0, :], ins["q8"][0, rsl, 0:LT])
            nc.sync.dma_start(qv0[:, j, 1, :], ins["v8"][0, rsl, 0:LT])
        for j in range(0, NR, 2):
            j2 = min(j + 2, NR)
            rsl = slice(j * P, j2 * P)
            nc.sync.dma_start(w1sb[:, j:j2, :],
                              ins["w1b"][rsl, :].rearrange("(nh p) o -> p nh o", p=P))
            nc.sync.dma_start(w2sb[:, j:j2, :],
                              ins["w2b"][rsl, :].rearrange("(nh p) o -> p nh o", p=P))
            nc.sync.dma_start(
                qb0[:, j:j2, :],
                ins["qb"][0, rsl, 0:LT].rearrange("(nh p) l -> p nh l", p=P))
            nc.sync.dma_start(
                vb0[:, j:j2, :],
                ins["vb16"][0, rsl, 0:LT].rearrange("(nh p) l -> p nh l", p=P))
        vws = consts.tile([P, NH, H], BF16)
        vws_loaded = []

        steps = [(b, lt) for b in range(BLOC) for lt in range(NLT)]
        vnat_tiles = {}

        def emit_stage_a(b, lt, preloaded=None):
            qv8, qsb, vsb = preloaded if preloaded is not None else load_qv(b, lt)

            # Stage A: hT[o, l] = tanh((W1' q'^T + W2' v'^T) * 2^-10 + b1 + b2)
            # fp8 DoubleRow blocks contract q_j AND v_j in one matmul.
            hT = hpool.tile([P, NH, LT], BF16, tag="hT")
            for o in range(NH):
                osl = slice(o * P, (o + 1) * P)
                pa = psA.tile([P, LT], F32, tag="pa")
                for j in range(NF8):
                    nc.tensor.matmul(pa, w128[:, j, :, osl], qv8[:, j, :, :],
                                     start=(j == 0), stop=(NR == 0 and j == NF8 - 1),
                                     perf_mode=DR)
                for j in range(NR):
                    nc.tensor.matmul(pa, w1sb[:, j, osl], qsb[:, j, :],
                                     start=(NF8 == 0 and j == 0), stop=False)
                    nc.tensor.matmul(pa, w2sb[:, j, osl], vsb[:, j, :],
                                     start=False, stop=(j == NR - 1))
                nc.scalar.activation(hT[:, o, :], pa, AFT.Tanh,
                                     bias=b12s[:, o:o + 1], scale=ASCALE)

            if not vws_loaded:
                for ht in range(NH):
                    nc.sync.dma_start(vws[:, ht, :],
                                      ins["vwt"][ht * P:(ht + 1) * P, :])
                vws_loaded.append(True)

            # value in natural [k, h] layout for the context matmul (used ~a full
            # step later, so the DMA is emitted after stage A's)
            if b not in vnat_tiles:
                vnat = vpool.tile([P, NH, H], BF16, tag="vnat")
                for j in range(0, NH, 2):
                    rsl = slice(j * P, (j + 2) * P)
                    nc.sync.dma_start(
                        vnat[:, j:j + 2, :],
                        ins["vn"][b, rsl, :].rearrange("(nk p) h -> p nk h", p=P))
                vnat_tiles.clear()
                vnat_tiles[b] = vnat
            vnat = vnat_tiles[b]
            return hT, vnat

        def emit_stage_b(b, lt, apart, last=False):
            hT, vnat = apart
            # Stage B: expT[o, l] = exp(Vw h + vb)  (no max-subtraction; scores
            # are small).
            # final tile: expT is f32r so the softmax sum can accumulate on
            # the PE right behind each exp, keeping the tail's DVE chain short
            expT = epool.tile([P, NH, LT], F32R if last else F32, tag="expT")
            ps = psS.tile([P, LT], F32, tag="ps")
            for o in range(NH):
                osl = slice(o * P, (o + 1) * P)
                pb = psB.tile([P, LT], F32, tag="pb")
                for ht in range(NH):
                    nc.tensor.matmul(pb, vws[:, ht, osl], hT[:, ht, :],
                                     start=(ht == 0), stop=(ht == NH - 1))
                nc.scalar.activation(expT[:, o, :], pb, AFT.Exp,
                                     bias=vbs[:, o:o + 1], scale=1.0)
                if last:
                    nc.tensor.matmul(ps, ones[:], expT[:, o, :],
                                     start=(o == 0), stop=(o == NH - 1))
            if not last:
                # Partition-dim softmax sums, replicated to all partitions:
                # DVE tree-sum over the 8 o-blocks, then one ones-matmul to
                # replicate the 128 partial sums across partitions.
                t4 = mpool.tile([P, 4, LT], F32, tag="t4")
                t2 = mpool.tile([P, 2, LT], F32, tag="t2")
                s1 = mpool.tile([P, LT], F32R, tag="s1")
                nc.vector.tensor_add(t4, expT[:, 0:4, :], expT[:, 4:8, :])
                nc.vector.tensor_add(t2, t4[:, 0:2, :], t4[:, 2:4, :])
                nc.vector.tensor_add(s1, t2[:, 0, :], t2[:, 1, :])
                nc.tensor.matmul(ps, ones[:], s1[:], start=True, stop=True)
            return (b, lt, expT, ps, vnat)

        def emit_softmax_context(state, last=False):
            b, lt, expT, ps, vnat = state
            lsl = slice(lt * LT, (lt + 1) * LT)
            # last tile's expT is f32r-typed; read through an f32 view, write
            # in-place through the f32r-typed AP (BIR writers-rounded rule)
            expf = expT[:].bitcast(F32) if last else expT
            recip = mpool.tile([P, LT], F32, tag="recip")
            rscr = mpool.tile([P, LT], F32, tag="rscr")
            # ~2 ULP, ~2.8x faster than reciprocal(); sums are ~1e3 so no edge cases
            nc.vector.reciprocal_approx_accurate(recip, ps, rscr)
            attw = apool.tile([P, NH, LT], BF16, tag="attw")
            # all bf16 attw muls first: they gate the context matmuls on PE.
            # On the last step (nothing left to hide the DVE chain behind) do
            # them in l-halves so the first context groups start sooner.
            halves = [slice(0, LT // 2), slice(LT // 2, LT)] if last \
                else [slice(0, LT)]
            for hsl2 in halves:
                for o in range(NH):
                    nc.vector.tensor_mul(attw[:, o, hsl2], expf[:, o, hsl2],
                                         recip[:, hsl2])
            for o in range(NH):
                nc.vector.tensor_mul(expT[:, o, :], expf[:, o, :], recip)
            nc.sync.dma_start(
                attT[b, :, lsl].rearrange("(nh p) l -> p nh l", p=P), expf)

            # Context: ctx[l, h] = sum_k att[k, l] * v[k, h]
            for lb in range(LT // P):
                row0 = lt * LT + lb * P
                for hti in range(NHT):
                    hsl = slice(hti * LT, (hti + 1) * LT)
                    pc = psC.tile([P, LT], F32, tag="pc")
                    for kt in range(NH):
                        nc.tensor.matmul(pc, attw[:, kt, lb * P:(lb + 1) * P],
                                         vnat[:, kt, hsl],
                                         start=(kt == 0), stop=(kt == NH - 1))
                    cs = cpool.tile([P, LT], BF16, tag="cs")
                    # PSUM->SBUF evacuation alternating ScalarE/DVE so neither
                    # queue's backlog blocks psC slot reuse for long
                    if hti == 0:
                        nc.scalar.activation(cs, pc, AFT.Copy)
                    else:
                        nc.vector.tensor_copy(cs, pc)
                    nc.sync.dma_start(ctxo[b, row0:row0 + P, hsl], cs)

        pending = None
        nsteps = len(steps)
        for i, (b, lt) in enumerate(steps):
            apart = emit_stage_a(
                b, lt, preloaded=(qv0, qb0, vb0) if i == 0 else None)
            state = emit_stage_b(b, lt, apart, last=(i == nsteps - 1))
            if pending is not None:
                emit_softmax_context(pending)
            pending = state
        emit_softmax_context(pending, last=True)


def _get_program():
    if "nc" not in _PROGRAM_CACHE:
        _PROGRAM_CACHE["nc"] = _build_program()
    return _PROGRAM_CACHE["nc"]


def _prep_in_maps(query, value, w1_w, w1_b, w2_w, w2_b, v_w, v_b):
    bf16 = ml_dtypes.bfloat16
    fp8 = ml_dtypes.float8_e4m3
    # [h, o] layouts; fp8 rows scaled by WS, bf16 rows too (exact power of
    # two) so the PSUM accumulator has one uniform scale.
    w1t = np.ascontiguousarray(w1_w.T) * WS
    w2t = np.ascontiguousarray(w2_w.T) * WS
    vwt = v_w.T.astype(bf16)
    b12 = np.ascontiguousarray((w1_b + w2_b).astype(np.float32).reshape(NH, P).T)
    vbt = np.ascontiguousarray(v_b.astype(np.float32).reshape(NH, P).T)

    base = {
        "vwt_in": vwt,
        "b12_in": b12,
        "vbt_in": vbt,
        "ones_in": np.ones((P, P), np.float32),
    }
    if NF8:
        base["w18_in"] = w1t[:NF8 * P].astype(fp8)
        base["w28_in"] = w2t[:NF8 * P].astype(fp8)
    if NR:
        base["w1b_in"] = w1t[NF8 * P:].astype(bf16)
        base["w2b_in"] = w2t[NF8 * P:].astype(bf16)

    in_maps = []
    for c in range(NCORES):
        sl = slice(c * BLOC, (c + 1) * BLOC)
        qT = query[sl].transpose(0, 2, 1) * QS
        vT = value[sl].transpose(0, 2, 1) * QS
        m = dict(base)
        if NF8:
            m["q8_in"] = qT[:, :NF8 * P].astype(fp8)
            m["v8_in"] = vT[:, :NF8 * P].astype(fp8)
        if NR:
            m["qb_in"] = qT[:, NF8 * P:].astype(bf16)
            m["vb16_in"] = vT[:, NF8 * P:].astype(bf16)
        m["vn_in"] = value[sl].astype(bf16)
        in_maps.append(m)
    return in_maps


def run_sharded(inputs, **run_kwargs):
    """Build in_maps, run on 8 cores, return (att, ctx, BassKernelResults)."""
    query = np.asarray(inputs["query"], dtype=np.float32)
    value = np.asarray(inputs["value"], dtype=np.float32)
    in_maps = _prep_in_maps(
        query, value,
        np.asarray(inputs["w1_w"], np.float32), np.asarray(inputs["w1_b"], np.float32),
        np.asarray(inputs["w2_w"], np.float32), np.asarray(inputs["w2_b"], np.float32),
        np.asarray(inputs["v_w"], np.float32), np.asarray(inputs["v_b"], np.float32),
    )
    nc = _get_program()
    res = bass_utils.run_bass_kernel_spmd(
        nc, in_maps, core_ids=list(range(NCORES)), **run_kwargs)

    att = np.empty((B, L, H), np.float32)
    ctxv = np.empty((B, L, H), np.float32)
    for c in range(NCORES):
        sl = slice(c * BLOC, (c + 1) * BLOC)
        att[sl] = res.results[c]["att_out"].transpose(0, 2, 1)
        ctxv[sl] = res.results[c]["ctx_out"].astype(np.float32)
    return att, ctxv, res


def kernel(**inputs):
    att, ctxv, _ = run_sharded(inputs)
    return att, ctxv
